# revision 12
# baseline (speedup 1.0000x reference)
"""Trainium2 Bass kernel for nn_DTFN (mass/flux stencil + vocab decoder).

Strategy (8 NeuronCores, SPMD single NEFF):
  - Sequence-parallel mass+flux: each core computes m for its S/8=256 seq
    positions using a K=3 halo of neighbors' initial state (no per-step
    exchange needed: halo depth K suffices for K local steps; global
    zero-flux boundaries handled by a per-core flux mask input).
  - AllGather the final m across the 8 cores, split into two halves so the
    decoder can start on the first half while the second is in flight.
  - Vocab-parallel decoder: each core computes logits for all 4096 tokens
    x its 4000-column shard of w_dec.
  All matmuls run as float32r (full PE rate at N>=256, ~13-bit mantissa).
  Layout is "transposed": d-chunks on partitions, tokens on the free axis,
  so flux-step outputs chain directly into the next matmul and into the
  decoder's stationary operand with no transposes after the initial one.
  State blocks are batch-major (e = b*DC + g) so each batch's flux update
  unblocks the next step's matmuls for that batch independently.
"""

import numpy as np
from contextlib import ExitStack

import concourse.bass as bass
import concourse.bacc as bacc
import concourse.mybir as mybir
import concourse.tile as tile
from concourse.bass_utils import run_bass_kernel_spmd
from concourse.masks import make_identity

F32 = mybir.dt.float32
F32R = mybir.dt.float32r
I32 = mybir.dt.int32
AF = mybir.ActivationFunctionType

V, D, KSTEPS, B, S = 32000, 512, 3, 2, 2048
EPS = 1e-6
NCORES = 8
SC = S // NCORES          # 256 seq positions per core
HALO = KSTEPS             # 3
WM = SC + 2 * HALO        # 262 m-cells per batch per core
WMA = WM + 2              # 264 allocated (2 zero-pad cells for even fp32r N)
WF = WM - 1               # 261 real flux pairs
WFP = WF + 1              # 262 computed pairs (fp32r needs even dst count)
NTOK = B * WM             # 524 gathered tokens per core
GT = (NTOK + 127) // 128  # 5 gather tiles (640 slots, rest padded)
DC = D // 128             # 4 d-chunks
E = DC * B                # 8 (batch, dchunk) blocks, b-major: e = b*DC + g
VS = V // NCORES          # 4000 vocab columns per core
NV = 8
VT = VS // NV             # 500
TJ = B * S // 128         # 32 token tiles of 128

_CACHE: dict = {}
LAST_RESULTS = None


def _build_module(variant="all"):
    nc = bacc.Bacc("TRN2", target_bir_lowering=False, debug=False,
                   num_devices=NCORES)

    do_flux = variant in ("all", "flux", "nocc2")
    do_dec = variant in ("all", "decoder", "nocc2")
    do_cc = variant in ("all", "cc")

    # --- per-core DRAM I/O ---
    t_idx = nc.dram_tensor("t_idx", [128, GT], I32, kind="ExternalInput")
    t_mdt = nc.dram_tensor("t_mdt", [128, WFP], F32, kind="ExternalInput")
    t_wm = nc.dram_tensor("t_wm", [128, DC * D], F32R, kind="ExternalInput")
    t_wf1 = nc.dram_tensor("t_wf1", [128, 2 * DC * D], F32R, kind="ExternalInput")
    t_wf2 = nc.dram_tensor("t_wf2", [128, DC * D], F32R, kind="ExternalInput")
    t_bm = nc.dram_tensor("t_bm", [128, DC], F32, kind="ExternalInput")
    t_bf1 = nc.dram_tensor("t_bf1", [128, DC], F32, kind="ExternalInput")
    t_bf2 = nc.dram_tensor("t_bf2", [128, DC], F32, kind="ExternalInput")
    t_emb = nc.dram_tensor("t_emb", [V, D], F32, kind="ExternalInput")
    t_wd = nc.dram_tensor("t_wd", [128, DC * VS], F32R, kind="ExternalInput")
    t_bd = nc.dram_tensor("t_bd", [128, VS], F32, kind="ExternalInput")
    t_y = nc.dram_tensor("t_y", [TJ * 128, VS], F32, kind="ExternalOutput")

    with tile.TileContext(nc) as tc:
        with ExitStack() as ctx:
            per = ctx.enter_context(tc.tile_pool(name="per", bufs=1))
            ps = ctx.enter_context(tc.tile_pool(name="ps", bufs=4, space="PSUM"))
            dram = ctx.enter_context(tc.tile_pool(name="dram", bufs=1, space="DRAM"))

            # ---- small loads first (so the gather isn't queued behind 10MB) ----
            idx_sb = per.tile([128, GT], I32)
            nc.sync.dma_start(idx_sb[:], t_idx.ap())
            mdt_sb = per.tile([128, WFP], F32)
            nc.sync.dma_start(mdt_sb[:], t_mdt.ap())
            bm_sb = per.tile([128, DC], F32)
            nc.sync.dma_start(bm_sb[:], t_bm.ap())
            bf1_sb = per.tile([128, DC], F32)
            nc.sync.dma_start(bf1_sb[:], t_bf1.ap())
            bf2_sb = per.tile([128, DC], F32)
            nc.sync.dma_start(bf2_sb[:], t_bf2.ap())
            wm_sb = per.tile([128, DC, D], F32R)
            nc.sync.dma_start(wm_sb[:], t_wm.ap().rearrange("p (g d) -> p g d", g=DC))
            wf1_sb = per.tile([128, 2, DC, D], F32R)
            wf2_sb = per.tile([128, DC, D], F32R)

            # persistent state: m transposed [p, (b g), t]
            mT = per.tile([128, E, WMA], F32R)
            # zero the 2 pad cells (memset can't emit f32r; mul-by-0 can)
            nc.vector.tensor_scalar_mul(
                mT[:, :, WM:WMA],
                mdt_sb[:, None, 0:2].to_broadcast([128, E, 2]), 0.0)
            mT_f32 = mT[:].bitcast(F32)
            # b-major views [p, g, b, t]
            mTv = mT[:].rearrange("p (b g) t -> p g b t", b=B)
            mTv_f32 = mT_f32.rearrange("p (b g) t -> p g b t", b=B)

            # ---- phase A: gather + transpose ----
            ctxa = ExitStack()
            if do_flux:
                pa = ctxa.enter_context(tc.tile_pool(name="pa", bufs=2))
                pe1 = ctxa.enter_context(tc.tile_pool(name="pe1", bufs=1))

                eT = pe1.tile([128, DC, GT * 128], F32R)
                ident = pe1.tile([128, 128], F32)
                make_identity(nc, ident[:])

                e_nats = []
                for gt in range(GT):
                    e_nat = pa.tile([128, D], F32, tag="e_nat", bufs=3)
                    nc.gpsimd.indirect_dma_start(
                        out=e_nat[:], out_offset=None,
                        in_=t_emb.ap(),
                        in_offset=bass.IndirectOffsetOnAxis(
                            ap=idx_sb[:, gt:gt + 1], axis=0),
                    )
                    e_nats.append(e_nat)

            if do_flux:
                for gt in range(GT):
                    for gd in range(DC):
                        tp = ps.tile([128, 2, 512], F32, tag="ps", space="PSUM")
                        nc.tensor.transpose(
                            tp[:, 0, 0:128],
                            e_nats[gt][:, gd * 128:(gd + 1) * 128],
                            ident[:])
                        nc.vector.tensor_copy(
                            eT[:, gd, gt * 128:(gt + 1) * 128], tp[:, 0, 0:128])

                # flux weights stream in while transposes/mass run
                nc.sync.dma_start(
                    wf1_sb[:],
                    t_wf1.ap().rearrange("p (s g d) -> p s g d", s=2, g=DC),
                )
                nc.sync.dma_start(
                    wf2_sb[:],
                    t_wf2.ap().rearrange("p (g d) -> p g d", g=DC))

                # mass: m = softplus(e @ w_mass + b_m)   [softplus = Ln(1+Exp)]
                for q in range(DC):
                    pm = ps.tile([128, 2, 512], F32, tag="ps", space="PSUM")
                    for gd in range(DC):
                        for bb in range(B):
                            nc.tensor.matmul(
                                pm[:, bb, 0:WM],
                                wm_sb[:, gd, q * 128:(q + 1) * 128],
                                eT[:, gd, bb * WM:(bb + 1) * WM],
                                start=(gd == 0), stop=(gd == DC - 1),
                            )
                    mtmp = pa.tile([128, B, WM], F32, tag="mtmp")
                    nc.scalar.activation(mtmp[:], pm[:, :, 0:WM], AF.Exp,
                                         bias=bm_sb[:, q:q + 1])
                    nc.scalar.activation(mTv[:, q, :, 0:WM], mtmp[:],
                                         AF.Ln, bias=1.0)
                ctxa.close()

            # big decoder weights here: stream during the flux steps
            wdec_sb = per.tile([128, DC * VS], F32R)
            bdec_sb = per.tile([128, VS], F32)
            if do_dec:
                nc.sync.dma_start(wdec_sb[:], t_wd.ap())
                nc.sync.dma_start(bdec_sb[:], t_bd.ap())

            # ---- phase B: K flux steps ----
            if do_flux:
                with ExitStack() as ctxb:
                    pb = ctxb.enter_context(tc.tile_pool(name="pb", bufs=2))
                    pfl = ctxb.enter_context(tc.tile_pool(name="pfl", bufs=1))
                    hT = pfl.tile([128, E, WFP], F32R)
                    F_sb = pfl.tile([128, E, WFP], F32)
                    G_sb = pfl.tile([128, E, WFP], F32)
                    Gd_sb = pfl.tile([128, E, WF - 1], F32)
                    mupd = pfl.tile([128, E, WM - 2], F32)
                    hTv = hT[:].rearrange("p (b g) t -> p g b t", b=B)
                    Fv = F_sb[:].rearrange("p (b g) t -> p g b t", b=B)
                    for k in range(KSTEPS):
                        # h = tanh(m_l @ W1a + m_r @ W1b + b1)
                        for q in range(DC):
                            ph = ps.tile([128, 2, 512], F32, tag="ps", space="PSUM")
                            for gd in range(DC):
                                for sh in range(2):
                                    for bb in range(B):
                                        nc.tensor.matmul(
                                            ph[:, bb, 0:WFP],
                                            wf1_sb[:, sh, gd, q * 128:(q + 1) * 128],
                                            mT[:, bb * DC + gd, sh:sh + WFP],
                                            start=(gd == 0 and sh == 0),
                                            stop=(gd == DC - 1 and sh == 1),
                                        )
                            nc.scalar.activation(hTv[:, q, :, :], ph[:, :, 0:WFP],
                                                 AF.Tanh, bias=bf1_sb[:, q:q + 1])
                        # F = softplus(h @ W2 + b2)
                        for q in range(DC):
                            pf = ps.tile([128, 2, 512], F32, tag="ps", space="PSUM")
                            for gd in range(DC):
                                for bb in range(B):
                                    nc.tensor.matmul(
                                        pf[:, bb, 0:WFP],
                                        wf2_sb[:, gd, q * 128:(q + 1) * 128],
                                        hT[:, bb * DC + gd, :],
                                        start=(gd == 0), stop=(gd == DC - 1),
                                    )
                            ftmp = pb.tile([128, B, WFP], F32, tag="ftmp")
                            nc.scalar.activation(ftmp[:], pf[:, :, 0:WFP], AF.Exp,
                                                 bias=bf2_sb[:, q:q + 1])
                            nc.scalar.activation(Fv[:, q, :, :], ftmp[:],
                                                 AF.Ln, bias=1.0)
                        # G = dt*mask*F ; dm = G[j-1]-G[j]; m = max(m+dm, EPS)
                        # per batch so each batch's next-step matmuls unblock early
                        for bb in range(B):
                            eb = bb * DC
                            nc.vector.tensor_tensor(
                                out=G_sb[:, eb:eb + DC, :],
                                in0=F_sb[:, eb:eb + DC, :],
                                in1=mdt_sb[:, None, :].to_broadcast([128, DC, WFP]),
                                op=mybir.AluOpType.mult,
                            )
                            nc.vector.tensor_tensor(
                                out=Gd_sb[:, eb:eb + DC, :],
                                in0=G_sb[:, eb:eb + DC, 0:WF - 1],
                                in1=G_sb[:, eb:eb + DC, 1:WF],
                                op=mybir.AluOpType.subtract,
                            )
                            nc.vector.tensor_tensor(
                                out=mupd[:, eb:eb + DC, :],
                                in0=mT_f32[:, eb:eb + DC, 1:WM - 1],
                                in1=Gd_sb[:, eb:eb + DC, :],
                                op=mybir.AluOpType.add,
                            )
                            nc.vector.tensor_scalar_max(
                                mT[:, eb:eb + DC, 1:WM - 1],
                                mupd[:, eb:eb + DC, :], EPS)

            # ---- phase C: allgather m, split into the two 128-token halves ----
            cc_outs = []
            for hh in range(2):
                cc_in = dram.tile([128, E * 128], F32, name=f"cc_in{hh}")
                cc_out = dram.tile([NCORES * 128, E * 128], F32, name=f"cc_out{hh}")
                if do_flux:
                    nc.sync.dma_start(
                        cc_in[:].rearrange("p (e t) -> p e t", e=E),
                        mT_f32[:, :, HALO + hh * 128: HALO + (hh + 1) * 128],
                    )
                elif variant == "cc":
                    nc.sync.dma_start(cc_in[:, 0:WFP], mdt_sb[:])
                if do_cc:
                    nc.gpsimd.collective_compute(
                        "AllGather", mybir.AluOpType.bypass,
                        replica_groups=[list(range(NCORES))],
                        ins=[cc_in[:]], outs=[cc_out[:]],
                    )
                cc_outs.append(cc_out[:].bitcast(F32R))

            # ---- phase D: decoder (h=0 token tiles first, then h=1) ----
            if do_dec:
                with ExitStack() as ctxd:
                    pl = ctxd.enter_context(tc.tile_pool(name="pl", bufs=8))
                    po = ctxd.enter_context(tc.tile_pool(name="po", bufs=2))
                    order = [j for j in range(TJ) if j % 2 == 0] + \
                            [j for j in range(TJ) if j % 2 == 1]
                    for j in order:
                        bb = j // 16
                        q16 = j % 16
                        r = q16 // 2
                        hh = q16 % 2
                        src = cc_outs[hh]
                        # all 4 d-chunks of this token tile are contiguous
                        lt = pl.tile([128, DC * 128], F32R, tag="lt")
                        nc.sync.dma_start(
                            lt[:],
                            src[r * 128:(r + 1) * 128,
                                bb * DC * 128:(bb + 1) * DC * 128],
                        )
                        osb = po.tile([128, VS], F32, tag="osb")
                        for v in range(NV):
                            pd = ps.tile([128, 2, 512], F32, tag="ps", space="PSUM")
                            for gd in range(DC):
                                nc.tensor.matmul(
                                    pd[:, 0, 0:VT],
                                    lt[:, gd * 128:(gd + 1) * 128],
                                    wdec_sb[:, gd * VS + v * VT:
                                            gd * VS + (v + 1) * VT],
                                    start=(gd == 0), stop=(gd == DC - 1),
                                )
                            nc.vector.tensor_tensor(
                                out=osb[:, v * VT:(v + 1) * VT],
                                in0=pd[:, 0, 0:VT],
                                in1=bdec_sb[:, v * VT:(v + 1) * VT],
                                op=mybir.AluOpType.add,
                            )
                        nc.sync.dma_start(
                            t_y.ap()[j * 128:(j + 1) * 128, :], osb[:])

    nc.compile()
    return nc


def _get_module(variant="all"):
    key = f"nc:{variant}"
    if key not in _CACHE:
        _CACHE[key] = _build_module(variant)
    return _CACHE[key]


def _prep_inputs(x, emb, w_mass, b_mass, w_f1, b_f1, w_f2, b_f2, cfl_raw,
                 w_dec, b_dec):
    x = np.asarray(x)
    emb = np.ascontiguousarray(np.asarray(emb, dtype=np.float32))
    w_mass = np.asarray(w_mass, dtype=np.float32)
    b_mass = np.asarray(b_mass, dtype=np.float32)
    w_f1 = np.asarray(w_f1, dtype=np.float32)
    b_f1 = np.asarray(b_f1, dtype=np.float32)
    w_f2 = np.asarray(w_f2, dtype=np.float32)
    b_f2 = np.asarray(b_f2, dtype=np.float32)
    w_dec = np.asarray(w_dec, dtype=np.float32)
    b_dec = np.asarray(b_dec, dtype=np.float32)
    dt = float(1.0 / (1.0 + np.exp(-np.float64(np.asarray(cfl_raw)))))

    wm_in = np.ascontiguousarray(
        w_mass.reshape(DC, 128, D).transpose(1, 0, 2).reshape(128, DC * D))
    wf1_in = np.ascontiguousarray(
        w_f1.reshape(2, DC, 128, D).transpose(2, 0, 1, 3).reshape(128, 2 * DC * D))
    wf2_in = np.ascontiguousarray(
        w_f2.reshape(DC, 128, D).transpose(1, 0, 2).reshape(128, DC * D))
    bm_in = np.ascontiguousarray(b_mass.reshape(DC, 128).T)
    bf1_in = np.ascontiguousarray(b_f1.reshape(DC, 128).T)
    bf2_in = np.ascontiguousarray(b_f2.reshape(DC, 128).T)

    in_maps = []
    for c in range(NCORES):
        sedge = c * SC - HALO
        idx = np.zeros(GT * 128, dtype=np.int32)
        for b in range(B):
            t = np.arange(WM)
            sc = np.clip(sedge + t, 0, S - 1)
            idx[b * WM:(b + 1) * WM] = x[b, sc]
        idx_t = np.ascontiguousarray(idx.reshape(GT, 128).T)

        j = np.arange(WFP)
        gp = sedge + j
        fm = ((gp >= 0) & (gp <= S - 2) & (j < WF)).astype(np.float32) \
            * np.float32(dt)
        mdt_in = np.ascontiguousarray(np.broadcast_to(fm, (128, WFP)))

        wd = w_dec[:, c * VS:(c + 1) * VS]
        wd_in = np.ascontiguousarray(
            wd.reshape(DC, 128, VS).transpose(1, 0, 2).reshape(128, DC * VS))
        bd_in = np.ascontiguousarray(
            np.broadcast_to(b_dec[c * VS:(c + 1) * VS], (128, VS)))

        in_maps.append({
            "t_idx": idx_t, "t_mdt": mdt_in,
            "t_wm": wm_in, "t_wf1": wf1_in, "t_wf2": wf2_in,
            "t_bm": bm_in, "t_bf1": bf1_in, "t_bf2": bf2_in,
            "t_emb": emb, "t_wd": wd_in, "t_bd": bd_in,
        })
    return in_maps


def kernel(**inputs) -> np.ndarray:
    global LAST_RESULTS
    nc = _get_module()
    in_maps = _prep_inputs(**inputs)
    res = run_bass_kernel_spmd(nc, in_maps, core_ids=list(range(NCORES)))
    LAST_RESULTS = res
    y = np.empty((B, S, V), dtype=np.float32)
    for c in range(NCORES):
        y[:, :, c * VS:(c + 1) * VS] = res.results[c]["t_y"].reshape(B, S, VS)
    return y


# revision 14
# speedup vs baseline: 5.8852x; 5.8852x over previous
"""Trainium2 Bass kernel for nn_DTFN (mass/flux stencil + vocab decoder).

Strategy (8 NeuronCores, SPMD single NEFF):
  - Sequence-parallel mass+flux: each core computes m for its S/8=256 seq
    positions using a K=3 halo of neighbors' initial state (no per-step
    exchange needed: halo depth K suffices for K local steps; global
    zero-flux boundaries handled by a per-core flux mask input).
  - AllGather the final m across the 8 cores, split into two halves so the
    decoder can start on the first half while the second is in flight.
  - Vocab-parallel decoder: each core computes logits for all 4096 tokens
    x its 4000-column shard of w_dec.
  All matmuls run as float32r (full PE rate at N>=256, ~13-bit mantissa).
  Layout is "transposed": d-chunks on partitions, tokens on the free axis,
  so flux-step outputs chain directly into the next matmul and into the
  decoder's stationary operand with no transposes after the initial one.
  State blocks are batch-major (e = b*DC + g) so each batch's flux update
  unblocks the next step's matmuls for that batch independently.
"""

import numpy as np
from contextlib import ExitStack

import concourse.bass as bass
import concourse.bacc as bacc
import concourse.mybir as mybir
import concourse.tile as tile
from concourse.bass_utils import run_bass_kernel_spmd
from concourse.masks import make_identity

F32 = mybir.dt.float32
F32R = mybir.dt.float32r
I32 = mybir.dt.int32
AF = mybir.ActivationFunctionType

V, D, KSTEPS, B, S = 32000, 512, 3, 2, 2048
EPS = 1e-6
NCORES = 8
SC = S // NCORES          # 256 seq positions per core
HALO = KSTEPS             # 3
WM = SC + 2 * HALO        # 262 m-cells per batch per core
WMA = WM + 2              # 264 allocated (2 zero-pad cells for even fp32r N)
WF = WM - 1               # 261 real flux pairs
WFP = WF + 1              # 262 computed pairs (fp32r needs even dst count)
NTOK = B * WM             # 524 gathered tokens per core
GT = (NTOK + 127) // 128  # 5 gather tiles (640 slots, rest padded)
DC = D // 128             # 4 d-chunks
E = DC * B                # 8 (batch, dchunk) blocks, b-major: e = b*DC + g
VS = V // NCORES          # 4000 vocab columns per core
NV = 8
VT = VS // NV             # 500
TJ = B * S // 128         # 32 token tiles of 128

_CACHE: dict = {}
LAST_RESULTS = None


def _build_module(variant="all"):
    nc = bacc.Bacc("TRN2", target_bir_lowering=False, debug=False,
                   num_devices=NCORES)

    do_flux = variant in ("all", "flux", "nocc2")
    do_dec = variant in ("all", "decoder", "nocc2")
    do_cc = variant in ("all", "cc")

    # --- per-core DRAM I/O ---
    t_idx = nc.dram_tensor("t_idx", [128, GT], I32, kind="ExternalInput")
    t_mdt = nc.dram_tensor("t_mdt", [128, WFP], F32, kind="ExternalInput")
    t_wm = nc.dram_tensor("t_wm", [128, DC * D], F32R, kind="ExternalInput")
    t_wf1 = nc.dram_tensor("t_wf1", [128, 2 * DC * D], F32R, kind="ExternalInput")
    t_wf2 = nc.dram_tensor("t_wf2", [128, DC * D], F32R, kind="ExternalInput")
    t_bm = nc.dram_tensor("t_bm", [128, DC], F32, kind="ExternalInput")
    t_bf1 = nc.dram_tensor("t_bf1", [128, DC], F32, kind="ExternalInput")
    t_bf2 = nc.dram_tensor("t_bf2", [128, DC], F32, kind="ExternalInput")
    t_emb = nc.dram_tensor("t_emb", [V, D], F32, kind="ExternalInput")
    t_wd = nc.dram_tensor("t_wd", [128, DC * VS], F32R, kind="ExternalInput")
    t_bd = nc.dram_tensor("t_bd", [128, VS], F32, kind="ExternalInput")
    t_y = nc.dram_tensor("t_y", [TJ * 128, VS], F32, kind="ExternalOutput")

    with tile.TileContext(nc) as tc:
        with ExitStack() as ctx:
            per = ctx.enter_context(tc.tile_pool(name="per", bufs=1))
            ps = ctx.enter_context(tc.tile_pool(name="ps", bufs=4, space="PSUM"))
            dram = ctx.enter_context(tc.tile_pool(name="dram", bufs=1, space="DRAM"))

            # ---- small loads first (so the gather isn't queued behind 10MB) ----
            idx_sb = per.tile([128, GT], I32)
            nc.sync.dma_start(idx_sb[:], t_idx.ap())
            mdt_sb = per.tile([128, WFP], F32)
            nc.sync.dma_start(mdt_sb[:], t_mdt.ap())
            bm_sb = per.tile([128, DC], F32)
            nc.sync.dma_start(bm_sb[:], t_bm.ap())
            bf1_sb = per.tile([128, DC], F32)
            nc.sync.dma_start(bf1_sb[:], t_bf1.ap())
            bf2_sb = per.tile([128, DC], F32)
            nc.sync.dma_start(bf2_sb[:], t_bf2.ap())
            wm_sb = per.tile([128, DC, D], F32R)
            nc.sync.dma_start(wm_sb[:], t_wm.ap().rearrange("p (g d) -> p g d", g=DC))
            wf1_sb = per.tile([128, 2, DC, D], F32R)
            wf2_sb = per.tile([128, DC, D], F32R)

            # persistent state: m transposed [p, (b g), t]
            mT = per.tile([128, E, WMA], F32R)
            # zero the 2 pad cells (memset can't emit f32r; mul-by-0 can)
            nc.vector.tensor_scalar_mul(
                mT[:, :, WM:WMA],
                mdt_sb[:, None, 0:2].to_broadcast([128, E, 2]), 0.0)
            mT_f32 = mT[:].bitcast(F32)
            # b-major views [p, g, b, t]
            mTv = mT[:].rearrange("p (b g) t -> p g b t", b=B)
            mTv_f32 = mT_f32.rearrange("p (b g) t -> p g b t", b=B)

            # ---- phase A: gather + transpose ----
            ctxa = ExitStack()
            if do_flux:
                pa = ctxa.enter_context(tc.tile_pool(name="pa", bufs=2))
                pe1 = ctxa.enter_context(tc.tile_pool(name="pe1", bufs=1))

                eT = pe1.tile([128, DC, GT * 128], F32R)
                ident = pe1.tile([128, 128], F32)
                make_identity(nc, ident[:])

                e_nats = []
                gather_insts = []
                for gt in range(GT):
                    e_nat = pa.tile([128, D], F32, tag="e_nat", bufs=3)
                    gi = nc.gpsimd.indirect_dma_start(
                        out=e_nat[:], out_offset=None,
                        in_=t_emb.ap(),
                        in_offset=bass.IndirectOffsetOnAxis(
                            ap=idx_sb[:, gt:gt + 1], axis=0),
                    )
                    gather_insts.append(gi)
                    e_nats.append(e_nat)

            if do_flux:
                for gt in range(GT):
                    for gd in range(DC):
                        tp = ps.tile([128, 2, 512], F32, tag="ps", space="PSUM")
                        nc.tensor.transpose(
                            tp[:, 0, 0:128],
                            e_nats[gt][:, gd * 128:(gd + 1) * 128],
                            ident[:])
                        nc.vector.tensor_copy(
                            eT[:, gd, gt * 128:(gt + 1) * 128], tp[:, 0, 0:128])

                # flux weights stream in while transposes/mass run
                nc.sync.dma_start(
                    wf1_sb[:],
                    t_wf1.ap().rearrange("p (s g d) -> p s g d", s=2, g=DC),
                )
                nc.sync.dma_start(
                    wf2_sb[:],
                    t_wf2.ap().rearrange("p (g d) -> p g d", g=DC))

                # mass: m = softplus(e @ w_mass + b_m)   [softplus = Ln(1+Exp)]
                for q in range(DC):
                    pm = ps.tile([128, 2, 512], F32, tag="ps", space="PSUM")
                    for gd in range(DC):
                        for bb in range(B):
                            nc.tensor.matmul(
                                pm[:, bb, 0:WM],
                                wm_sb[:, gd, q * 128:(q + 1) * 128],
                                eT[:, gd, bb * WM:(bb + 1) * WM],
                                start=(gd == 0), stop=(gd == DC - 1),
                            )
                    mtmp = pa.tile([128, B, WM], F32, tag="mtmp")
                    nc.scalar.activation(mtmp[:], pm[:, :, 0:WM], AF.Exp,
                                         bias=bm_sb[:, q:q + 1])
                    nc.scalar.activation(mTv[:, q, :, 0:WM], mtmp[:],
                                         AF.Ln, bias=1.0)
                ctxa.close()

            # big decoder weights here: stream during the flux steps
            wdec_sb = per.tile([128, DC * VS], F32R)
            bdec_sb = per.tile([128, VS], F32)
            if do_dec:
                wd_i = nc.sync.dma_start(wdec_sb[:], t_wd.ap())
                nc.sync.dma_start(bdec_sb[:], t_bd.ap())
                if do_flux:
                    # keep the big stream out of the gathers' way
                    from concourse.bass import _add_dep_helper
                    _add_dep_helper(wd_i.ins, gather_insts[-1].ins, sync=True,
                                    reason="wdec stream after gathers")

            # ---- phase B: K flux steps ----
            if do_flux:
                with ExitStack() as ctxb:
                    pb = ctxb.enter_context(tc.tile_pool(name="pb", bufs=2))
                    pfl = ctxb.enter_context(tc.tile_pool(name="pfl", bufs=1))
                    hT = pfl.tile([128, E, WFP], F32R)
                    F_sb = pfl.tile([128, E, WFP], F32)
                    G_sb = pfl.tile([128, E, WFP], F32)
                    Gd_sb = pfl.tile([128, E, WF - 1], F32)
                    mupd = pfl.tile([128, E, WM - 2], F32)
                    hTv = hT[:].rearrange("p (b g) t -> p g b t", b=B)
                    Fv = F_sb[:].rearrange("p (b g) t -> p g b t", b=B)
                    for k in range(KSTEPS):
                        # h = tanh(m_l @ W1a + m_r @ W1b + b1)
                        for q in range(DC):
                            ph = ps.tile([128, 2, 512], F32, tag="ps", space="PSUM")
                            for gd in range(DC):
                                for sh in range(2):
                                    for bb in range(B):
                                        nc.tensor.matmul(
                                            ph[:, bb, 0:WFP],
                                            wf1_sb[:, sh, gd, q * 128:(q + 1) * 128],
                                            mT[:, bb * DC + gd, sh:sh + WFP],
                                            start=(gd == 0 and sh == 0),
                                            stop=(gd == DC - 1 and sh == 1),
                                        )
                            nc.scalar.activation(hTv[:, q, :, :], ph[:, :, 0:WFP],
                                                 AF.Tanh, bias=bf1_sb[:, q:q + 1])
                        # F = softplus(h @ W2 + b2)
                        for q in range(DC):
                            pf = ps.tile([128, 2, 512], F32, tag="ps", space="PSUM")
                            for gd in range(DC):
                                for bb in range(B):
                                    nc.tensor.matmul(
                                        pf[:, bb, 0:WFP],
                                        wf2_sb[:, gd, q * 128:(q + 1) * 128],
                                        hT[:, bb * DC + gd, :],
                                        start=(gd == 0), stop=(gd == DC - 1),
                                    )
                            ftmp = pb.tile([128, B, WFP], F32, tag="ftmp")
                            nc.scalar.activation(ftmp[:], pf[:, :, 0:WFP], AF.Exp,
                                                 bias=bf2_sb[:, q:q + 1])
                            nc.scalar.activation(Fv[:, q, :, :], ftmp[:],
                                                 AF.Ln, bias=1.0)
                        # G = dt*mask*F ; dm = G[j-1]-G[j]; m = max(m+dm, EPS)
                        # split per (batch, d-chunk): the update is elementwise
                        # over d, so each chunk's chain unblocks its next-step
                        # matmuls as soon as that chunk's F is ready
                        for bb in range(B):
                            for q in range(DC):
                                e0 = bb * DC + q
                                nc.vector.tensor_tensor(
                                    out=G_sb[:, e0:e0 + 1, :],
                                    in0=F_sb[:, e0:e0 + 1, :],
                                    in1=mdt_sb[:, None, :],
                                    op=mybir.AluOpType.mult,
                                )
                                nc.vector.tensor_tensor(
                                    out=Gd_sb[:, e0:e0 + 1, :],
                                    in0=G_sb[:, e0:e0 + 1, 0:WF - 1],
                                    in1=G_sb[:, e0:e0 + 1, 1:WF],
                                    op=mybir.AluOpType.subtract,
                                )
                                nc.vector.tensor_tensor(
                                    out=mupd[:, e0:e0 + 1, :],
                                    in0=mT_f32[:, e0:e0 + 1, 1:WM - 1],
                                    in1=Gd_sb[:, e0:e0 + 1, :],
                                    op=mybir.AluOpType.add,
                                )
                                nc.vector.tensor_scalar_max(
                                    mT[:, e0:e0 + 1, 1:WM - 1],
                                    mupd[:, e0:e0 + 1, :], EPS)

            # ---- phase C: allgather m, split into the two 128-token halves ----
            cc_outs = []
            for hh in range(2):
                cc_in = dram.tile([128, E * 128], F32, name=f"cc_in{hh}")
                cc_out = dram.tile([NCORES * 128, E * 128], F32, name=f"cc_out{hh}")
                if do_flux:
                    nc.sync.dma_start(
                        cc_in[:].rearrange("p (e t) -> p e t", e=E),
                        mT_f32[:, :, HALO + hh * 128: HALO + (hh + 1) * 128],
                    )
                elif variant == "cc":
                    nc.sync.dma_start(cc_in[:, 0:WFP], mdt_sb[:])
                if do_cc:
                    nc.gpsimd.collective_compute(
                        "AllGather", mybir.AluOpType.bypass,
                        replica_groups=[list(range(NCORES))],
                        ins=[cc_in[:]], outs=[cc_out[:]],
                    )
                cc_outs.append(cc_out[:].bitcast(F32R))

            # ---- phase D: decoder (h=0 token tiles first, then h=1) ----
            if do_dec:
                with ExitStack() as ctxd:
                    pl = ctxd.enter_context(tc.tile_pool(name="pl", bufs=8))
                    po = ctxd.enter_context(tc.tile_pool(name="po", bufs=2))
                    order = [j for j in range(TJ) if j % 2 == 0] + \
                            [j for j in range(TJ) if j % 2 == 1]
                    for j in order:
                        bb = j // 16
                        q16 = j % 16
                        r = q16 // 2
                        hh = q16 % 2
                        src = cc_outs[hh]
                        # all 4 d-chunks of this token tile are contiguous
                        lt = pl.tile([128, DC * 128], F32R, tag="lt")
                        nc.sync.dma_start(
                            lt[:],
                            src[r * 128:(r + 1) * 128,
                                bb * DC * 128:(bb + 1) * DC * 128],
                        )
                        osb = po.tile([128, VS], F32, tag="osb")
                        for v in range(NV):
                            pd = ps.tile([128, 2, 512], F32, tag="ps", space="PSUM")
                            for gd in range(DC):
                                nc.tensor.matmul(
                                    pd[:, 0, 0:VT],
                                    lt[:, gd * 128:(gd + 1) * 128],
                                    wdec_sb[:, gd * VS + v * VT:
                                            gd * VS + (v + 1) * VT],
                                    start=(gd == 0), stop=(gd == DC - 1),
                                )
                            nc.vector.tensor_tensor(
                                out=osb[:, v * VT:(v + 1) * VT],
                                in0=pd[:, 0, 0:VT],
                                in1=bdec_sb[:, v * VT:(v + 1) * VT],
                                op=mybir.AluOpType.add,
                            )
                        nc.sync.dma_start(
                            t_y.ap()[j * 128:(j + 1) * 128, :], osb[:])

    nc.compile()
    return nc


def _get_module(variant="all"):
    key = f"nc:{variant}"
    if key not in _CACHE:
        _CACHE[key] = _build_module(variant)
    return _CACHE[key]


def _prep_inputs(x, emb, w_mass, b_mass, w_f1, b_f1, w_f2, b_f2, cfl_raw,
                 w_dec, b_dec):
    x = np.asarray(x)
    emb = np.ascontiguousarray(np.asarray(emb, dtype=np.float32))
    w_mass = np.asarray(w_mass, dtype=np.float32)
    b_mass = np.asarray(b_mass, dtype=np.float32)
    w_f1 = np.asarray(w_f1, dtype=np.float32)
    b_f1 = np.asarray(b_f1, dtype=np.float32)
    w_f2 = np.asarray(w_f2, dtype=np.float32)
    b_f2 = np.asarray(b_f2, dtype=np.float32)
    w_dec = np.asarray(w_dec, dtype=np.float32)
    b_dec = np.asarray(b_dec, dtype=np.float32)
    dt = float(1.0 / (1.0 + np.exp(-np.float64(np.asarray(cfl_raw)))))

    wm_in = np.ascontiguousarray(
        w_mass.reshape(DC, 128, D).transpose(1, 0, 2).reshape(128, DC * D))
    wf1_in = np.ascontiguousarray(
        w_f1.reshape(2, DC, 128, D).transpose(2, 0, 1, 3).reshape(128, 2 * DC * D))
    wf2_in = np.ascontiguousarray(
        w_f2.reshape(DC, 128, D).transpose(1, 0, 2).reshape(128, DC * D))
    bm_in = np.ascontiguousarray(b_mass.reshape(DC, 128).T)
    bf1_in = np.ascontiguousarray(b_f1.reshape(DC, 128).T)
    bf2_in = np.ascontiguousarray(b_f2.reshape(DC, 128).T)

    in_maps = []
    for c in range(NCORES):
        sedge = c * SC - HALO
        idx = np.zeros(GT * 128, dtype=np.int32)
        for b in range(B):
            t = np.arange(WM)
            sc = np.clip(sedge + t, 0, S - 1)
            idx[b * WM:(b + 1) * WM] = x[b, sc]
        idx_t = np.ascontiguousarray(idx.reshape(GT, 128).T)

        j = np.arange(WFP)
        gp = sedge + j
        fm = ((gp >= 0) & (gp <= S - 2) & (j < WF)).astype(np.float32) \
            * np.float32(dt)
        mdt_in = np.ascontiguousarray(np.broadcast_to(fm, (128, WFP)))

        wd = w_dec[:, c * VS:(c + 1) * VS]
        wd_in = np.ascontiguousarray(
            wd.reshape(DC, 128, VS).transpose(1, 0, 2).reshape(128, DC * VS))
        bd_in = np.ascontiguousarray(
            np.broadcast_to(b_dec[c * VS:(c + 1) * VS], (128, VS)))

        in_maps.append({
            "t_idx": idx_t, "t_mdt": mdt_in,
            "t_wm": wm_in, "t_wf1": wf1_in, "t_wf2": wf2_in,
            "t_bm": bm_in, "t_bf1": bf1_in, "t_bf2": bf2_in,
            "t_emb": emb, "t_wd": wd_in, "t_bd": bd_in,
        })
    return in_maps


def kernel(**inputs) -> np.ndarray:
    global LAST_RESULTS
    nc = _get_module()
    in_maps = _prep_inputs(**inputs)
    res = run_bass_kernel_spmd(nc, in_maps, core_ids=list(range(NCORES)))
    LAST_RESULTS = res
    y = np.empty((B, S, V), dtype=np.float32)
    for c in range(NCORES):
        y[:, :, c * VS:(c + 1) * VS] = res.results[c]["t_y"].reshape(B, S, VS)
    return y


# revision 16
# speedup vs baseline: 5.9236x; 1.0065x over previous
"""Trainium2 Bass kernel for nn_DTFN (mass/flux stencil + vocab decoder).

Strategy (8 NeuronCores, SPMD single NEFF):
  - Sequence-parallel mass+flux: each core computes m for its S/8=256 seq
    positions using a K=3 halo of neighbors' initial state (no per-step
    exchange needed: halo depth K suffices for K local steps; global
    zero-flux boundaries handled by a per-core flux mask input).
  - AllGather the final m across the 8 cores, split into two halves so the
    decoder can start on the first half while the second is in flight.
  - Vocab-parallel decoder: each core computes logits for all 4096 tokens
    x its 4000-column shard of w_dec.
  All matmuls run as float32r (full PE rate at N>=256, ~13-bit mantissa).
  Layout is "transposed": d-chunks on partitions, tokens on the free axis,
  so flux-step outputs chain directly into the next matmul and into the
  decoder's stationary operand with no transposes after the initial one.
  State blocks are batch-major (e = b*DC + g) so each batch's flux update
  unblocks the next step's matmuls for that batch independently.
"""

import numpy as np
from contextlib import ExitStack

import concourse.bass as bass
import concourse.bacc as bacc
import concourse.mybir as mybir
import concourse.tile as tile
from concourse.bass_utils import run_bass_kernel_spmd
from concourse.masks import make_identity

F32 = mybir.dt.float32
F32R = mybir.dt.float32r
I32 = mybir.dt.int32
AF = mybir.ActivationFunctionType

V, D, KSTEPS, B, S = 32000, 512, 3, 2, 2048
EPS = 1e-6
NCORES = 8
SC = S // NCORES          # 256 seq positions per core
HALO = KSTEPS             # 3
WM = SC + 2 * HALO        # 262 m-cells per batch per core
WMA = WM + 2              # 264 allocated (2 zero-pad cells for even fp32r N)
WF = WM - 1               # 261 real flux pairs
WFP = WF + 1              # 262 computed pairs (fp32r needs even dst count)
NTOK = B * WM             # 524 gathered tokens per core
GT = (NTOK + 127) // 128  # 5 gather tiles (640 slots, rest padded)
DC = D // 128             # 4 d-chunks
E = DC * B                # 8 (batch, dchunk) blocks, b-major: e = b*DC + g
VS = V // NCORES          # 4000 vocab columns per core
NV = 8
VT = VS // NV             # 500
TJ = B * S // 128         # 32 token tiles of 128

_CACHE: dict = {}
LAST_RESULTS = None


def _build_module(variant="all"):
    nc = bacc.Bacc("TRN2", target_bir_lowering=False, debug=False,
                   num_devices=NCORES)

    do_flux = variant in ("all", "flux", "nocc2")
    do_dec = variant in ("all", "decoder", "nocc2")
    do_cc = variant in ("all", "cc")

    # --- per-core DRAM I/O ---
    t_idx = nc.dram_tensor("t_idx", [128, GT], I32, kind="ExternalInput")
    t_mdt = nc.dram_tensor("t_mdt", [128, WFP], F32, kind="ExternalInput")
    t_wm = nc.dram_tensor("t_wm", [128, DC * D], F32R, kind="ExternalInput")
    t_wf1 = nc.dram_tensor("t_wf1", [128, 2 * DC * D], F32R, kind="ExternalInput")
    t_wf2 = nc.dram_tensor("t_wf2", [128, DC * D], F32R, kind="ExternalInput")
    t_bm = nc.dram_tensor("t_bm", [128, DC], F32, kind="ExternalInput")
    t_bf1 = nc.dram_tensor("t_bf1", [128, DC], F32, kind="ExternalInput")
    t_bf2 = nc.dram_tensor("t_bf2", [128, DC], F32, kind="ExternalInput")
    t_emb = nc.dram_tensor("t_emb", [V, D], F32, kind="ExternalInput")
    t_wd = nc.dram_tensor("t_wd", [128, DC * VS], F32R, kind="ExternalInput")
    t_bd = nc.dram_tensor("t_bd", [128, VS], F32, kind="ExternalInput")
    t_y = nc.dram_tensor("t_y", [TJ * 128, VS], F32, kind="ExternalOutput")

    with tile.TileContext(nc) as tc:
        with ExitStack() as ctx:
            per = ctx.enter_context(tc.tile_pool(name="per", bufs=1))
            dram = ctx.enter_context(tc.tile_pool(name="dram", bufs=1, space="DRAM"))

            # ---- small loads first (so the gather isn't queued behind 10MB) ----
            idx_sb = per.tile([128, GT], I32)
            nc.sync.dma_start(idx_sb[:], t_idx.ap())
            mdt_sb = per.tile([128, WFP], F32)
            nc.sync.dma_start(mdt_sb[:], t_mdt.ap())
            bm_sb = per.tile([128, DC], F32)
            nc.sync.dma_start(bm_sb[:], t_bm.ap())
            bf1_sb = per.tile([128, DC], F32)
            nc.sync.dma_start(bf1_sb[:], t_bf1.ap())
            bf2_sb = per.tile([128, DC], F32)
            nc.sync.dma_start(bf2_sb[:], t_bf2.ap())
            wm_sb = per.tile([128, DC, D], F32R)
            nc.sync.dma_start(wm_sb[:], t_wm.ap().rearrange("p (g d) -> p g d", g=DC))
            wf1_sb = per.tile([128, 2, DC, D], F32R)
            wf2_sb = per.tile([128, DC, D], F32R)

            # persistent state: m transposed [p, (b g), t]
            mT = per.tile([128, E, WMA], F32R)
            # zero the 2 pad cells (memset can't emit f32r; mul-by-0 can)
            nc.vector.tensor_scalar_mul(
                mT[:, :, WM:WMA],
                mdt_sb[:, None, 0:2].to_broadcast([128, E, 2]), 0.0)
            mT_f32 = mT[:].bitcast(F32)
            # b-major views [p, g, b, t]
            mTv = mT[:].rearrange("p (b g) t -> p g b t", b=B)
            mTv_f32 = mT_f32.rearrange("p (b g) t -> p g b t", b=B)

            # ---- phase A: gather + transpose ----
            ctxa = ExitStack()
            ctxf = ExitStack()
            if do_flux:
                psf = ctxf.enter_context(
                    tc.tile_pool(name="psf", bufs=4, space="PSUM"))
                pa = ctxa.enter_context(tc.tile_pool(name="pa", bufs=2))
                pe1 = ctxa.enter_context(tc.tile_pool(name="pe1", bufs=1))

                eT = pe1.tile([128, DC, GT * 128], F32R)
                ident = pe1.tile([128, 128], F32)
                make_identity(nc, ident[:])

                e_nats = []
                gather_insts = []
                for gt in range(GT):
                    e_nat = pa.tile([128, D], F32, tag="e_nat", bufs=3)
                    gi = nc.gpsimd.indirect_dma_start(
                        out=e_nat[:], out_offset=None,
                        in_=t_emb.ap(),
                        in_offset=bass.IndirectOffsetOnAxis(
                            ap=idx_sb[:, gt:gt + 1], axis=0),
                    )
                    gather_insts.append(gi)
                    e_nats.append(e_nat)

            if do_flux:
                for gt in range(GT):
                    for gd in range(DC):
                        tp = psf.tile([128, 2, 512], F32, tag="ps", space="PSUM")
                        nc.tensor.transpose(
                            tp[:, 0, 0:128],
                            e_nats[gt][:, gd * 128:(gd + 1) * 128],
                            ident[:])
                        nc.vector.tensor_copy(
                            eT[:, gd, gt * 128:(gt + 1) * 128], tp[:, 0, 0:128])

                # flux weights stream in while transposes/mass run
                nc.sync.dma_start(
                    wf1_sb[:],
                    t_wf1.ap().rearrange("p (s g d) -> p s g d", s=2, g=DC),
                )
                nc.sync.dma_start(
                    wf2_sb[:],
                    t_wf2.ap().rearrange("p (g d) -> p g d", g=DC))

                # mass: m = softplus(e @ w_mass + b_m)   [softplus = Ln(1+Exp)]
                for q in range(DC):
                    pm = psf.tile([128, 2, 512], F32, tag="ps", space="PSUM")
                    for gd in range(DC):
                        for bb in range(B):
                            nc.tensor.matmul(
                                pm[:, bb, 0:WM],
                                wm_sb[:, gd, q * 128:(q + 1) * 128],
                                eT[:, gd, bb * WM:(bb + 1) * WM],
                                start=(gd == 0), stop=(gd == DC - 1),
                            )
                    mtmp = pa.tile([128, B, WM], F32, tag="mtmp")
                    nc.scalar.activation(mtmp[:], pm[:, :, 0:WM], AF.Exp,
                                         bias=bm_sb[:, q:q + 1])
                    nc.scalar.activation(mTv[:, q, :, 0:WM], mtmp[:],
                                         AF.Ln, bias=1.0)
                ctxa.close()

            # big decoder weights here: stream during the flux steps
            wdec_sb = per.tile([128, DC * VS], F32R)
            bdec_sb = per.tile([128, VS], F32)
            if do_dec:
                wd_i = nc.sync.dma_start(wdec_sb[:], t_wd.ap())
                nc.sync.dma_start(bdec_sb[:], t_bd.ap())
                if do_flux:
                    # keep the big stream out of the gathers' way
                    from concourse.bass import _add_dep_helper
                    _add_dep_helper(wd_i.ins, gather_insts[-1].ins, sync=True,
                                    reason="wdec stream after gathers")

            # ---- phase B: K flux steps ----
            if do_flux:
                with ExitStack() as ctxb:
                    pb = ctxb.enter_context(tc.tile_pool(name="pb", bufs=2))
                    pfl = ctxb.enter_context(tc.tile_pool(name="pfl", bufs=1))
                    hT = pfl.tile([128, E, WFP], F32R)
                    F_sb = pfl.tile([128, E, WFP], F32)
                    G_sb = pfl.tile([128, E, WFP], F32)
                    Gd_sb = pfl.tile([128, E, WF - 1], F32)
                    mupd = pfl.tile([128, E, WM - 2], F32)
                    hTv = hT[:].rearrange("p (b g) t -> p g b t", b=B)
                    Fv = F_sb[:].rearrange("p (b g) t -> p g b t", b=B)
                    for k in range(KSTEPS):
                        # h = tanh(m_l @ W1a + m_r @ W1b + b1)
                        for q in range(DC):
                            ph = psf.tile([128, 2, 512], F32, tag="ps", space="PSUM")
                            for gd in range(DC):
                                for sh in range(2):
                                    for bb in range(B):
                                        nc.tensor.matmul(
                                            ph[:, bb, 0:WFP],
                                            wf1_sb[:, sh, gd, q * 128:(q + 1) * 128],
                                            mT[:, bb * DC + gd, sh:sh + WFP],
                                            start=(gd == 0 and sh == 0),
                                            stop=(gd == DC - 1 and sh == 1),
                                        )
                            nc.scalar.activation(hTv[:, q, :, :], ph[:, :, 0:WFP],
                                                 AF.Tanh, bias=bf1_sb[:, q:q + 1])
                        # F = softplus(h @ W2 + b2)
                        for q in range(DC):
                            pf = psf.tile([128, 2, 512], F32, tag="ps", space="PSUM")
                            for gd in range(DC):
                                for bb in range(B):
                                    nc.tensor.matmul(
                                        pf[:, bb, 0:WFP],
                                        wf2_sb[:, gd, q * 128:(q + 1) * 128],
                                        hT[:, bb * DC + gd, :],
                                        start=(gd == 0), stop=(gd == DC - 1),
                                    )
                            ftmp = pb.tile([128, B, WFP], F32, tag="ftmp")
                            nc.scalar.activation(ftmp[:], pf[:, :, 0:WFP], AF.Exp,
                                                 bias=bf2_sb[:, q:q + 1])
                            nc.scalar.activation(Fv[:, q, :, :], ftmp[:],
                                                 AF.Ln, bias=1.0)
                        # G = dt*mask*F ; dm = G[j-1]-G[j]; m = max(m+dm, EPS)
                        # split per (batch, d-chunk): the update is elementwise
                        # over d, so each chunk's chain unblocks its next-step
                        # matmuls as soon as that chunk's F is ready
                        for bb in range(B):
                            for q in range(DC):
                                e0 = bb * DC + q
                                nc.vector.tensor_tensor(
                                    out=G_sb[:, e0:e0 + 1, :],
                                    in0=F_sb[:, e0:e0 + 1, :],
                                    in1=mdt_sb[:, None, :],
                                    op=mybir.AluOpType.mult,
                                )
                                nc.vector.tensor_tensor(
                                    out=Gd_sb[:, e0:e0 + 1, :],
                                    in0=G_sb[:, e0:e0 + 1, 0:WF - 1],
                                    in1=G_sb[:, e0:e0 + 1, 1:WF],
                                    op=mybir.AluOpType.subtract,
                                )
                                nc.vector.tensor_tensor(
                                    out=mupd[:, e0:e0 + 1, :],
                                    in0=mT_f32[:, e0:e0 + 1, 1:WM - 1],
                                    in1=Gd_sb[:, e0:e0 + 1, :],
                                    op=mybir.AluOpType.add,
                                )
                                nc.vector.tensor_scalar_max(
                                    mT[:, e0:e0 + 1, 1:WM - 1],
                                    mupd[:, e0:e0 + 1, :], EPS)

            ctxf.close()

            # ---- phase C: allgather m, split into the two 128-token halves ----
            cc_outs = []
            for hh in range(2):
                cc_in = dram.tile([128, E * 128], F32, name=f"cc_in{hh}")
                cc_out = dram.tile([NCORES * 128, E * 128], F32, name=f"cc_out{hh}")
                if do_flux:
                    nc.sync.dma_start(
                        cc_in[:].rearrange("p (e t) -> p e t", e=E),
                        mT_f32[:, :, HALO + hh * 128: HALO + (hh + 1) * 128],
                    )
                elif variant == "cc":
                    nc.sync.dma_start(cc_in[:, 0:WFP], mdt_sb[:])
                if do_cc:
                    nc.gpsimd.collective_compute(
                        "AllGather", mybir.AluOpType.bypass,
                        replica_groups=[list(range(NCORES))],
                        ins=[cc_in[:]], outs=[cc_out[:]],
                    )
                cc_outs.append(cc_out[:].bitcast(F32R))

            # ---- phase D: decoder (h=0 token tiles first, then h=1) ----
            if do_dec:
                with ExitStack() as ctxd:
                    psd_pool = ctxd.enter_context(
                        tc.tile_pool(name="psd", bufs=8, space="PSUM"))
                    pl = ctxd.enter_context(tc.tile_pool(name="pl", bufs=8))
                    po = ctxd.enter_context(tc.tile_pool(name="po", bufs=2))
                    order = [j for j in range(TJ) if j % 2 == 0] + \
                            [j for j in range(TJ) if j % 2 == 1]
                    for j in order:
                        bb = j // 16
                        q16 = j % 16
                        r = q16 // 2
                        hh = q16 % 2
                        src = cc_outs[hh]
                        # all 4 d-chunks of this token tile are contiguous
                        lt = pl.tile([128, DC * 128], F32R, tag="lt")
                        nc.sync.dma_start(
                            lt[:],
                            src[r * 128:(r + 1) * 128,
                                bb * DC * 128:(bb + 1) * DC * 128],
                        )
                        osb = po.tile([128, VS], F32, tag="osb")
                        for v in range(NV):
                            pd = psd_pool.tile([128, 512], F32, tag="psd")
                            for gd in range(DC):
                                nc.tensor.matmul(
                                    pd[:, 0:VT],
                                    lt[:, gd * 128:(gd + 1) * 128],
                                    wdec_sb[:, gd * VS + v * VT:
                                            gd * VS + (v + 1) * VT],
                                    start=(gd == 0), stop=(gd == DC - 1),
                                )
                            nc.vector.tensor_tensor(
                                out=osb[:, v * VT:(v + 1) * VT],
                                in0=pd[:, 0:VT],
                                in1=bdec_sb[:, v * VT:(v + 1) * VT],
                                op=mybir.AluOpType.add,
                            )
                        nc.sync.dma_start(
                            t_y.ap()[j * 128:(j + 1) * 128, :], osb[:])

    nc.compile()
    return nc


def _get_module(variant="all"):
    key = f"nc:{variant}"
    if key not in _CACHE:
        _CACHE[key] = _build_module(variant)
    return _CACHE[key]


def _prep_inputs(x, emb, w_mass, b_mass, w_f1, b_f1, w_f2, b_f2, cfl_raw,
                 w_dec, b_dec):
    x = np.asarray(x)
    emb = np.ascontiguousarray(np.asarray(emb, dtype=np.float32))
    w_mass = np.asarray(w_mass, dtype=np.float32)
    b_mass = np.asarray(b_mass, dtype=np.float32)
    w_f1 = np.asarray(w_f1, dtype=np.float32)
    b_f1 = np.asarray(b_f1, dtype=np.float32)
    w_f2 = np.asarray(w_f2, dtype=np.float32)
    b_f2 = np.asarray(b_f2, dtype=np.float32)
    w_dec = np.asarray(w_dec, dtype=np.float32)
    b_dec = np.asarray(b_dec, dtype=np.float32)
    dt = float(1.0 / (1.0 + np.exp(-np.float64(np.asarray(cfl_raw)))))

    wm_in = np.ascontiguousarray(
        w_mass.reshape(DC, 128, D).transpose(1, 0, 2).reshape(128, DC * D))
    wf1_in = np.ascontiguousarray(
        w_f1.reshape(2, DC, 128, D).transpose(2, 0, 1, 3).reshape(128, 2 * DC * D))
    wf2_in = np.ascontiguousarray(
        w_f2.reshape(DC, 128, D).transpose(1, 0, 2).reshape(128, DC * D))
    bm_in = np.ascontiguousarray(b_mass.reshape(DC, 128).T)
    bf1_in = np.ascontiguousarray(b_f1.reshape(DC, 128).T)
    bf2_in = np.ascontiguousarray(b_f2.reshape(DC, 128).T)

    in_maps = []
    for c in range(NCORES):
        sedge = c * SC - HALO
        idx = np.zeros(GT * 128, dtype=np.int32)
        for b in range(B):
            t = np.arange(WM)
            sc = np.clip(sedge + t, 0, S - 1)
            idx[b * WM:(b + 1) * WM] = x[b, sc]
        idx_t = np.ascontiguousarray(idx.reshape(GT, 128).T)

        j = np.arange(WFP)
        gp = sedge + j
        fm = ((gp >= 0) & (gp <= S - 2) & (j < WF)).astype(np.float32) \
            * np.float32(dt)
        mdt_in = np.ascontiguousarray(np.broadcast_to(fm, (128, WFP)))

        wd = w_dec[:, c * VS:(c + 1) * VS]
        wd_in = np.ascontiguousarray(
            wd.reshape(DC, 128, VS).transpose(1, 0, 2).reshape(128, DC * VS))
        bd_in = np.ascontiguousarray(
            np.broadcast_to(b_dec[c * VS:(c + 1) * VS], (128, VS)))

        in_maps.append({
            "t_idx": idx_t, "t_mdt": mdt_in,
            "t_wm": wm_in, "t_wf1": wf1_in, "t_wf2": wf2_in,
            "t_bm": bm_in, "t_bf1": bf1_in, "t_bf2": bf2_in,
            "t_emb": emb, "t_wd": wd_in, "t_bd": bd_in,
        })
    return in_maps


def kernel(**inputs) -> np.ndarray:
    global LAST_RESULTS
    nc = _get_module()
    in_maps = _prep_inputs(**inputs)
    res = run_bass_kernel_spmd(nc, in_maps, core_ids=list(range(NCORES)))
    LAST_RESULTS = res
    y = np.empty((B, S, V), dtype=np.float32)
    for c in range(NCORES):
        y[:, :, c * VS:(c + 1) * VS] = res.results[c]["t_y"].reshape(B, S, VS)
    return y


# revision 18
# speedup vs baseline: 6.0907x; 1.0282x over previous
"""Trainium2 Bass kernel for nn_DTFN (mass/flux stencil + vocab decoder).

Strategy (8 NeuronCores, SPMD single NEFF):
  - Sequence-parallel mass+flux: each core computes m for its S/8=256 seq
    positions using a K=3 halo of neighbors' initial state (no per-step
    exchange needed: halo depth K suffices for K local steps; global
    zero-flux boundaries handled by a per-core flux mask input).
  - AllGather the final m across the 8 cores, split into two halves so the
    decoder can start on the first half while the second is in flight.
  - Vocab-parallel decoder: each core computes logits for all 4096 tokens
    x its 4000-column shard of w_dec.
  All matmuls run as float32r (full PE rate at N>=256, ~13-bit mantissa).
  Layout is "transposed": d-chunks on partitions, tokens on the free axis,
  so flux-step outputs chain directly into the next matmul and into the
  decoder's stationary operand with no transposes after the initial one.
  State blocks are batch-major (e = b*DC + g) so each batch's flux update
  unblocks the next step's matmuls for that batch independently.
"""

import numpy as np
from contextlib import ExitStack

import concourse.bass as bass
import concourse.bacc as bacc
import concourse.mybir as mybir
import concourse.tile as tile
from concourse.bass_utils import run_bass_kernel_spmd
from concourse.masks import make_identity

F32 = mybir.dt.float32
F32R = mybir.dt.float32r
I32 = mybir.dt.int32
AF = mybir.ActivationFunctionType

V, D, KSTEPS, B, S = 32000, 512, 3, 2, 2048
EPS = 1e-6
NCORES = 8
SC = S // NCORES          # 256 seq positions per core
HALO = KSTEPS             # 3
WM = SC + 2 * HALO        # 262 m-cells per batch per core
WMA = WM + 2              # 264 allocated (2 zero-pad cells for even fp32r N)
WF = WM - 1               # 261 real flux pairs
WFP = WF + 1              # 262 computed pairs (fp32r needs even dst count)
NTOK = B * WM             # 524 gathered tokens per core
GT = (NTOK + 127) // 128  # 5 gather tiles (640 slots, rest padded)
DC = D // 128             # 4 d-chunks
E = DC * B                # 8 (batch, dchunk) blocks, b-major: e = b*DC + g
VS = V // NCORES          # 4000 vocab columns per core
NV = 8
VT = VS // NV             # 500
TJ = B * S // 128         # 32 token tiles of 128

_CACHE: dict = {}
LAST_RESULTS = None


def _build_module(variant="all"):
    nc = bacc.Bacc("TRN2", target_bir_lowering=False, debug=False,
                   num_devices=NCORES)

    do_flux = variant in ("all", "flux", "nocc2")
    do_dec = variant in ("all", "decoder", "nocc2")
    do_cc = variant in ("all", "cc")

    # --- per-core DRAM I/O ---
    t_idx = nc.dram_tensor("t_idx", [128, GT], I32, kind="ExternalInput")
    t_mdt = nc.dram_tensor("t_mdt", [128, WFP], F32, kind="ExternalInput")
    t_wm = nc.dram_tensor("t_wm", [128, DC * D], F32R, kind="ExternalInput")
    t_wf1 = nc.dram_tensor("t_wf1", [128, 2 * DC * D], F32R, kind="ExternalInput")
    t_wf2 = nc.dram_tensor("t_wf2", [128, DC * D], F32R, kind="ExternalInput")
    t_bm = nc.dram_tensor("t_bm", [128, DC], F32, kind="ExternalInput")
    t_bf1 = nc.dram_tensor("t_bf1", [128, DC], F32, kind="ExternalInput")
    t_bf2 = nc.dram_tensor("t_bf2", [128, DC], F32, kind="ExternalInput")
    t_emb = nc.dram_tensor("t_emb", [V, D], F32, kind="ExternalInput")
    t_wd = nc.dram_tensor("t_wd", [128, DC * VS], F32R, kind="ExternalInput")
    t_bd = nc.dram_tensor("t_bd", [128, VS], F32, kind="ExternalInput")
    t_y = nc.dram_tensor("t_y", [TJ * 128, VS], F32, kind="ExternalOutput")

    with tile.TileContext(nc) as tc:
        with ExitStack() as ctx:
            per = ctx.enter_context(tc.tile_pool(name="per", bufs=1))
            dram = ctx.enter_context(tc.tile_pool(name="dram", bufs=1, space="DRAM"))
            ctxw = ExitStack()
            pw = ctxw.enter_context(tc.tile_pool(name="pw", bufs=1))

            # ---- small loads first (so the gather isn't queued behind 10MB) ----
            idx_sb = pw.tile([128, GT], I32)
            nc.sync.dma_start(idx_sb[:], t_idx.ap())
            mdt_sb = pw.tile([128, WFP], F32)
            nc.sync.dma_start(mdt_sb[:], t_mdt.ap())
            bm_sb = pw.tile([128, DC], F32)
            nc.sync.dma_start(bm_sb[:], t_bm.ap())
            bf1_sb = pw.tile([128, DC], F32)
            nc.sync.dma_start(bf1_sb[:], t_bf1.ap())
            bf2_sb = pw.tile([128, DC], F32)
            nc.sync.dma_start(bf2_sb[:], t_bf2.ap())
            wm_sb = pw.tile([128, DC, D], F32R)
            nc.sync.dma_start(wm_sb[:], t_wm.ap().rearrange("p (g d) -> p g d", g=DC))
            wf1_sb = pw.tile([128, 2, DC, D], F32R)
            wf2_sb = pw.tile([128, DC, D], F32R)

            # persistent state: m transposed [p, (b g), t]
            mT = pw.tile([128, E, WMA], F32R)
            # zero the 2 pad cells (memset can't emit f32r; mul-by-0 can)
            nc.vector.tensor_scalar_mul(
                mT[:, :, WM:WMA],
                mdt_sb[:, None, 0:2].to_broadcast([128, E, 2]), 0.0)
            mT_f32 = mT[:].bitcast(F32)
            # b-major views [p, g, b, t]
            mTv = mT[:].rearrange("p (b g) t -> p g b t", b=B)
            mTv_f32 = mT_f32.rearrange("p (b g) t -> p g b t", b=B)

            # ---- phase A: gather + transpose ----
            ctxa = ExitStack()
            ctxf = ExitStack()
            if do_flux:
                psf = ctxf.enter_context(
                    tc.tile_pool(name="psf", bufs=4, space="PSUM"))
                pa = ctxa.enter_context(tc.tile_pool(name="pa", bufs=2))
                pe1 = ctxa.enter_context(tc.tile_pool(name="pe1", bufs=1))

                eT = pe1.tile([128, DC, GT * 128], F32R)
                ident = pe1.tile([128, 128], F32)
                make_identity(nc, ident[:])

                e_nats = []
                gather_insts = []
                for gt in range(GT):
                    e_nat = pa.tile([128, D], F32, tag="e_nat", bufs=3)
                    gi = nc.gpsimd.indirect_dma_start(
                        out=e_nat[:], out_offset=None,
                        in_=t_emb.ap(),
                        in_offset=bass.IndirectOffsetOnAxis(
                            ap=idx_sb[:, gt:gt + 1], axis=0),
                    )
                    gather_insts.append(gi)
                    e_nats.append(e_nat)

            if do_flux:
                for gt in range(GT):
                    for gd in range(DC):
                        tp = psf.tile([128, 2, 512], F32, tag="ps", space="PSUM")
                        nc.tensor.transpose(
                            tp[:, 0, 0:128],
                            e_nats[gt][:, gd * 128:(gd + 1) * 128],
                            ident[:])
                        nc.vector.tensor_copy(
                            eT[:, gd, gt * 128:(gt + 1) * 128], tp[:, 0, 0:128])

                # flux weights stream in while transposes/mass run
                nc.sync.dma_start(
                    wf1_sb[:],
                    t_wf1.ap().rearrange("p (s g d) -> p s g d", s=2, g=DC),
                )
                nc.sync.dma_start(
                    wf2_sb[:],
                    t_wf2.ap().rearrange("p (g d) -> p g d", g=DC))

                # mass: m = softplus(e @ w_mass + b_m)   [softplus = Ln(1+Exp)]
                for q in range(DC):
                    pm = psf.tile([128, 2, 512], F32, tag="ps", space="PSUM")
                    for gd in range(DC):
                        for bb in range(B):
                            nc.tensor.matmul(
                                pm[:, bb, 0:WM],
                                wm_sb[:, gd, q * 128:(q + 1) * 128],
                                eT[:, gd, bb * WM:(bb + 1) * WM],
                                start=(gd == 0), stop=(gd == DC - 1),
                            )
                    mtmp = pa.tile([128, B, WM], F32, tag="mtmp")
                    nc.scalar.activation(mtmp[:], pm[:, :, 0:WM], AF.Exp,
                                         bias=bm_sb[:, q:q + 1])
                    nc.scalar.activation(mTv[:, q, :, 0:WM], mtmp[:],
                                         AF.Ln, bias=1.0)
                ctxa.close()

            # big decoder weights here: stream during the flux steps
            wdec_sb = per.tile([128, DC * VS], F32R)
            bdec_sb = per.tile([128, VS], F32)
            if do_dec:
                wd_i = nc.sync.dma_start(wdec_sb[:], t_wd.ap())
                nc.sync.dma_start(bdec_sb[:], t_bd.ap())
                if do_flux:
                    # keep the big stream out of the gathers' way
                    from concourse.bass import _add_dep_helper
                    _add_dep_helper(wd_i.ins, gather_insts[-1].ins, sync=True,
                                    reason="wdec stream after gathers")

            # ---- phase B: K flux steps ----
            if do_flux:
                with ExitStack() as ctxb:
                    pb = ctxb.enter_context(tc.tile_pool(name="pb", bufs=2))
                    pfl = ctxb.enter_context(tc.tile_pool(name="pfl", bufs=1))
                    hT = pfl.tile([128, E, WFP], F32R)
                    F_sb = pfl.tile([128, E, WFP], F32)
                    G_sb = pfl.tile([128, E, WFP], F32)
                    Gd_sb = pfl.tile([128, E, WF - 1], F32)
                    mupd = pfl.tile([128, E, WM - 2], F32)
                    hTv = hT[:].rearrange("p (b g) t -> p g b t", b=B)
                    Fv = F_sb[:].rearrange("p (b g) t -> p g b t", b=B)
                    for k in range(KSTEPS):
                        # h = tanh(m_l @ W1a + m_r @ W1b + b1)
                        for q in range(DC):
                            ph = psf.tile([128, 2, 512], F32, tag="ps", space="PSUM")
                            for gd in range(DC):
                                for sh in range(2):
                                    for bb in range(B):
                                        nc.tensor.matmul(
                                            ph[:, bb, 0:WFP],
                                            wf1_sb[:, sh, gd, q * 128:(q + 1) * 128],
                                            mT[:, bb * DC + gd, sh:sh + WFP],
                                            start=(gd == 0 and sh == 0),
                                            stop=(gd == DC - 1 and sh == 1),
                                        )
                            nc.scalar.activation(hTv[:, q, :, :], ph[:, :, 0:WFP],
                                                 AF.Tanh, bias=bf1_sb[:, q:q + 1])
                        # F = softplus(h @ W2 + b2)
                        for q in range(DC):
                            pf = psf.tile([128, 2, 512], F32, tag="ps", space="PSUM")
                            for gd in range(DC):
                                for bb in range(B):
                                    nc.tensor.matmul(
                                        pf[:, bb, 0:WFP],
                                        wf2_sb[:, gd, q * 128:(q + 1) * 128],
                                        hT[:, bb * DC + gd, :],
                                        start=(gd == 0), stop=(gd == DC - 1),
                                    )
                            ftmp = pb.tile([128, B, WFP], F32, tag="ftmp")
                            nc.scalar.activation(ftmp[:], pf[:, :, 0:WFP], AF.Exp,
                                                 bias=bf2_sb[:, q:q + 1])
                            nc.scalar.activation(Fv[:, q, :, :], ftmp[:],
                                                 AF.Ln, bias=1.0)
                        # G = dt*mask*F ; dm = G[j-1]-G[j]; m = max(m+dm, EPS)
                        # split per (batch, d-chunk): the update is elementwise
                        # over d, so each chunk's chain unblocks its next-step
                        # matmuls as soon as that chunk's F is ready
                        for bb in range(B):
                            for q in range(DC):
                                e0 = bb * DC + q
                                nc.vector.tensor_tensor(
                                    out=G_sb[:, e0:e0 + 1, :],
                                    in0=F_sb[:, e0:e0 + 1, :],
                                    in1=mdt_sb[:, None, :],
                                    op=mybir.AluOpType.mult,
                                )
                                nc.vector.tensor_tensor(
                                    out=Gd_sb[:, e0:e0 + 1, :],
                                    in0=G_sb[:, e0:e0 + 1, 0:WF - 1],
                                    in1=G_sb[:, e0:e0 + 1, 1:WF],
                                    op=mybir.AluOpType.subtract,
                                )
                                nc.vector.tensor_tensor(
                                    out=mupd[:, e0:e0 + 1, :],
                                    in0=mT_f32[:, e0:e0 + 1, 1:WM - 1],
                                    in1=Gd_sb[:, e0:e0 + 1, :],
                                    op=mybir.AluOpType.add,
                                )
                                nc.vector.tensor_scalar_max(
                                    mT[:, e0:e0 + 1, 1:WM - 1],
                                    mupd[:, e0:e0 + 1, :], EPS)

            ctxf.close()

            # ---- phase C: allgather m, split into the two 128-token halves ----
            cc_outs = []
            for hh in range(2):
                cc_in = dram.tile([128, E * 128], F32, name=f"cc_in{hh}")
                cc_out = dram.tile([NCORES * 128, E * 128], F32, name=f"cc_out{hh}")
                if do_flux:
                    nc.sync.dma_start(
                        cc_in[:].rearrange("p (e t) -> p e t", e=E),
                        mT_f32[:, :, HALO + hh * 128: HALO + (hh + 1) * 128],
                    )
                elif variant == "cc":
                    nc.sync.dma_start(cc_in[:, 0:WFP], mdt_sb[:])
                if do_cc:
                    nc.gpsimd.collective_compute(
                        "AllGather", mybir.AluOpType.bypass,
                        replica_groups=[list(range(NCORES))],
                        ins=[cc_in[:]], outs=[cc_out[:]],
                    )
                cc_outs.append(cc_out[:].bitcast(F32R))
            ctxw.close()

            # ---- phase D: decoder (h=0 token tiles first, then h=1) ----
            if do_dec:
                with ExitStack() as ctxd:
                    psd_pool = ctxd.enter_context(
                        tc.tile_pool(name="psd", bufs=8, space="PSUM"))
                    pl = ctxd.enter_context(tc.tile_pool(name="pl", bufs=8))
                    po = ctxd.enter_context(tc.tile_pool(name="po", bufs=4))
                    order = [j for j in range(TJ) if j % 2 == 0] + \
                            [j for j in range(TJ) if j % 2 == 1]
                    for j in order:
                        bb = j // 16
                        q16 = j % 16
                        r = q16 // 2
                        hh = q16 % 2
                        src = cc_outs[hh]
                        # all 4 d-chunks of this token tile are contiguous
                        lt = pl.tile([128, DC * 128], F32R, tag="lt")
                        nc.sync.dma_start(
                            lt[:],
                            src[r * 128:(r + 1) * 128,
                                bb * DC * 128:(bb + 1) * DC * 128],
                        )
                        osb = po.tile([128, VS], F32, tag="osb")
                        for v in range(NV):
                            pd = psd_pool.tile([128, 512], F32, tag="psd")
                            for gd in range(DC):
                                nc.tensor.matmul(
                                    pd[:, 0:VT],
                                    lt[:, gd * 128:(gd + 1) * 128],
                                    wdec_sb[:, gd * VS + v * VT:
                                            gd * VS + (v + 1) * VT],
                                    start=(gd == 0), stop=(gd == DC - 1),
                                )
                            nc.vector.tensor_tensor(
                                out=osb[:, v * VT:(v + 1) * VT],
                                in0=pd[:, 0:VT],
                                in1=bdec_sb[:, v * VT:(v + 1) * VT],
                                op=mybir.AluOpType.add,
                            )
                        nc.sync.dma_start(
                            t_y.ap()[j * 128:(j + 1) * 128, :], osb[:])

    nc.compile()
    return nc


def _get_module(variant="all"):
    key = f"nc:{variant}"
    if key not in _CACHE:
        _CACHE[key] = _build_module(variant)
    return _CACHE[key]


def _prep_inputs(x, emb, w_mass, b_mass, w_f1, b_f1, w_f2, b_f2, cfl_raw,
                 w_dec, b_dec):
    x = np.asarray(x)
    emb = np.ascontiguousarray(np.asarray(emb, dtype=np.float32))
    w_mass = np.asarray(w_mass, dtype=np.float32)
    b_mass = np.asarray(b_mass, dtype=np.float32)
    w_f1 = np.asarray(w_f1, dtype=np.float32)
    b_f1 = np.asarray(b_f1, dtype=np.float32)
    w_f2 = np.asarray(w_f2, dtype=np.float32)
    b_f2 = np.asarray(b_f2, dtype=np.float32)
    w_dec = np.asarray(w_dec, dtype=np.float32)
    b_dec = np.asarray(b_dec, dtype=np.float32)
    dt = float(1.0 / (1.0 + np.exp(-np.float64(np.asarray(cfl_raw)))))

    wm_in = np.ascontiguousarray(
        w_mass.reshape(DC, 128, D).transpose(1, 0, 2).reshape(128, DC * D))
    wf1_in = np.ascontiguousarray(
        w_f1.reshape(2, DC, 128, D).transpose(2, 0, 1, 3).reshape(128, 2 * DC * D))
    wf2_in = np.ascontiguousarray(
        w_f2.reshape(DC, 128, D).transpose(1, 0, 2).reshape(128, DC * D))
    bm_in = np.ascontiguousarray(b_mass.reshape(DC, 128).T)
    bf1_in = np.ascontiguousarray(b_f1.reshape(DC, 128).T)
    bf2_in = np.ascontiguousarray(b_f2.reshape(DC, 128).T)

    in_maps = []
    for c in range(NCORES):
        sedge = c * SC - HALO
        idx = np.zeros(GT * 128, dtype=np.int32)
        for b in range(B):
            t = np.arange(WM)
            sc = np.clip(sedge + t, 0, S - 1)
            idx[b * WM:(b + 1) * WM] = x[b, sc]
        idx_t = np.ascontiguousarray(idx.reshape(GT, 128).T)

        j = np.arange(WFP)
        gp = sedge + j
        fm = ((gp >= 0) & (gp <= S - 2) & (j < WF)).astype(np.float32) \
            * np.float32(dt)
        mdt_in = np.ascontiguousarray(np.broadcast_to(fm, (128, WFP)))

        wd = w_dec[:, c * VS:(c + 1) * VS]
        wd_in = np.ascontiguousarray(
            wd.reshape(DC, 128, VS).transpose(1, 0, 2).reshape(128, DC * VS))
        bd_in = np.ascontiguousarray(
            np.broadcast_to(b_dec[c * VS:(c + 1) * VS], (128, VS)))

        in_maps.append({
            "t_idx": idx_t, "t_mdt": mdt_in,
            "t_wm": wm_in, "t_wf1": wf1_in, "t_wf2": wf2_in,
            "t_bm": bm_in, "t_bf1": bf1_in, "t_bf2": bf2_in,
            "t_emb": emb, "t_wd": wd_in, "t_bd": bd_in,
        })
    return in_maps


def kernel(**inputs) -> np.ndarray:
    global LAST_RESULTS
    nc = _get_module()
    in_maps = _prep_inputs(**inputs)
    res = run_bass_kernel_spmd(nc, in_maps, core_ids=list(range(NCORES)))
    LAST_RESULTS = res
    y = np.empty((B, S, V), dtype=np.float32)
    for c in range(NCORES):
        y[:, :, c * VS:(c + 1) * VS] = res.results[c]["t_y"].reshape(B, S, VS)
    return y


# revision 19
# speedup vs baseline: 6.5627x; 1.0775x over previous
"""Trainium2 Bass kernel for nn_DTFN (mass/flux stencil + vocab decoder).

Strategy (8 NeuronCores, SPMD single NEFF):
  - Sequence-parallel mass+flux: each core computes m for its S/8=256 seq
    positions using a K=3 halo of neighbors' initial state (no per-step
    exchange needed: halo depth K suffices for K local steps; global
    zero-flux boundaries handled by a per-core flux mask input).
  - AllGather the final m across the 8 cores, split into two halves so the
    decoder can start on the first half while the second is in flight.
  - Vocab-parallel decoder: each core computes logits for all 4096 tokens
    x its 4000-column shard of w_dec.
  All matmuls run as float32r (full PE rate at N>=256, ~13-bit mantissa).
  Layout is "transposed": d-chunks on partitions, tokens on the free axis,
  so flux-step outputs chain directly into the next matmul and into the
  decoder's stationary operand with no transposes after the initial one.
  State blocks are batch-major (e = b*DC + g) so each batch's flux update
  unblocks the next step's matmuls for that batch independently.
"""

import numpy as np
from contextlib import ExitStack

import concourse.bass as bass
import concourse.bacc as bacc
import concourse.mybir as mybir
import concourse.tile as tile
from concourse.bass_utils import run_bass_kernel_spmd
from concourse.masks import make_identity

F32 = mybir.dt.float32
F32R = mybir.dt.float32r
I32 = mybir.dt.int32
AF = mybir.ActivationFunctionType

V, D, KSTEPS, B, S = 32000, 512, 3, 2, 2048
EPS = 1e-6
NCORES = 8
SC = S // NCORES          # 256 seq positions per core
HALO = KSTEPS             # 3
WM = SC + 2 * HALO        # 262 m-cells per batch per core
WMA = WM + 2              # 264 allocated (2 zero-pad cells for even fp32r N)
WF = WM - 1               # 261 real flux pairs
WFP = WF + 1              # 262 computed pairs (fp32r needs even dst count)
NTOK = B * WM             # 524 gathered tokens per core
GT = (NTOK + 127) // 128  # 5 gather tiles (640 slots, rest padded)
DC = D // 128             # 4 d-chunks
E = DC * B                # 8 (batch, dchunk) blocks, b-major: e = b*DC + g
VS = V // NCORES          # 4000 vocab columns per core
NV = 8
VT = VS // NV             # 500
TJ = B * S // 128         # 32 token tiles of 128

_CACHE: dict = {}
LAST_RESULTS = None


def _build_module(variant="all"):
    nc = bacc.Bacc("TRN2", target_bir_lowering=False, debug=False,
                   num_devices=NCORES)

    do_flux = variant in ("all", "flux", "nocc2")
    do_dec = variant in ("all", "decoder", "nocc2")
    do_cc = variant in ("all", "cc")

    # --- per-core DRAM I/O ---
    t_idx = nc.dram_tensor("t_idx", [128, GT], I32, kind="ExternalInput")
    t_mdt = nc.dram_tensor("t_mdt", [128, WFP], F32, kind="ExternalInput")
    t_wm = nc.dram_tensor("t_wm", [128, DC * D], F32R, kind="ExternalInput")
    t_wf1 = nc.dram_tensor("t_wf1", [128, 2 * DC * D], F32R, kind="ExternalInput")
    t_wf2 = nc.dram_tensor("t_wf2", [128, DC * D], F32R, kind="ExternalInput")
    t_bm = nc.dram_tensor("t_bm", [128, DC], F32, kind="ExternalInput")
    t_bf1 = nc.dram_tensor("t_bf1", [128, DC], F32, kind="ExternalInput")
    t_bf2 = nc.dram_tensor("t_bf2", [128, DC], F32, kind="ExternalInput")
    t_emb = nc.dram_tensor("t_emb", [V, D], F32, kind="ExternalInput")
    t_wd = nc.dram_tensor("t_wd", [128, DC * VS], F32R, kind="ExternalInput")
    t_bd = nc.dram_tensor("t_bd", [128, VS], F32, kind="ExternalInput")
    t_y = nc.dram_tensor("t_y", [TJ * 128, VS], F32, kind="ExternalOutput")

    with tile.TileContext(nc) as tc:
        with ExitStack() as ctx:
            per = ctx.enter_context(tc.tile_pool(name="per", bufs=1))
            dram = ctx.enter_context(tc.tile_pool(name="dram", bufs=1, space="DRAM"))
            ctxw = ExitStack()
            pw = ctxw.enter_context(tc.tile_pool(name="pw", bufs=1))

            # ---- small loads first (so the gather isn't queued behind 10MB) ----
            idx_sb = pw.tile([128, GT], I32)
            nc.sync.dma_start(idx_sb[:], t_idx.ap())
            mdt_sb = pw.tile([128, WFP], F32)
            nc.sync.dma_start(mdt_sb[:], t_mdt.ap())
            bm_sb = pw.tile([128, DC], F32)
            nc.sync.dma_start(bm_sb[:], t_bm.ap())
            bf1_sb = pw.tile([128, DC], F32)
            nc.sync.dma_start(bf1_sb[:], t_bf1.ap())
            bf2_sb = pw.tile([128, DC], F32)
            nc.sync.dma_start(bf2_sb[:], t_bf2.ap())
            wm_sb = pw.tile([128, DC, D], F32R)
            nc.sync.dma_start(wm_sb[:], t_wm.ap().rearrange("p (g d) -> p g d", g=DC))
            wf1_sb = pw.tile([128, 2, DC, D], F32R)
            wf2_sb = pw.tile([128, DC, D], F32R)

            # persistent state: m transposed [p, (b g), t]
            mT = pw.tile([128, E, WMA], F32R)
            # zero the 2 pad cells (memset can't emit f32r; mul-by-0 can)
            nc.vector.tensor_scalar_mul(
                mT[:, :, WM:WMA],
                mdt_sb[:, None, 0:2].to_broadcast([128, E, 2]), 0.0)
            mT_f32 = mT[:].bitcast(F32)
            # b-major views [p, g, b, t]
            mTv = mT[:].rearrange("p (b g) t -> p g b t", b=B)
            mTv_f32 = mT_f32.rearrange("p (b g) t -> p g b t", b=B)

            # ---- phase A: gather + transpose ----
            ctxa = ExitStack()
            ctxf = ExitStack()
            if do_flux:
                psf = ctxf.enter_context(
                    tc.tile_pool(name="psf", bufs=4, space="PSUM"))
                pa = ctxa.enter_context(tc.tile_pool(name="pa", bufs=2))
                pe1 = ctxa.enter_context(tc.tile_pool(name="pe1", bufs=1))

                eT = pe1.tile([128, DC, GT * 128], F32R)
                ident = pe1.tile([128, 128], F32)
                make_identity(nc, ident[:])

                e_nats = []
                gather_insts = []
                for gt in range(GT):
                    e_nat = pa.tile([128, D], F32, tag="e_nat", bufs=3)
                    gi = nc.gpsimd.indirect_dma_start(
                        out=e_nat[:], out_offset=None,
                        in_=t_emb.ap(),
                        in_offset=bass.IndirectOffsetOnAxis(
                            ap=idx_sb[:, gt:gt + 1], axis=0),
                    )
                    gather_insts.append(gi)
                    e_nats.append(e_nat)

            if do_flux:
                for gt in range(GT):
                    for gd in range(DC):
                        tp = psf.tile([128, 2, 512], F32, tag="ps", space="PSUM")
                        nc.tensor.transpose(
                            tp[:, 0, 0:128],
                            e_nats[gt][:, gd * 128:(gd + 1) * 128],
                            ident[:])
                        nc.vector.tensor_copy(
                            eT[:, gd, gt * 128:(gt + 1) * 128], tp[:, 0, 0:128])

                # flux weights stream in while transposes/mass run
                nc.sync.dma_start(
                    wf1_sb[:],
                    t_wf1.ap().rearrange("p (s g d) -> p s g d", s=2, g=DC),
                )
                nc.sync.dma_start(
                    wf2_sb[:],
                    t_wf2.ap().rearrange("p (g d) -> p g d", g=DC))

                # mass: m = softplus(e @ w_mass + b_m)   [softplus = Ln(1+Exp)]
                for q in range(DC):
                    pm = psf.tile([128, 2, 512], F32, tag="ps", space="PSUM")
                    for gd in range(DC):
                        for bb in range(B):
                            nc.tensor.matmul(
                                pm[:, bb, 0:WM],
                                wm_sb[:, gd, q * 128:(q + 1) * 128],
                                eT[:, gd, bb * WM:(bb + 1) * WM],
                                start=(gd == 0), stop=(gd == DC - 1),
                            )
                    mtmp = pa.tile([128, B, WM], F32, tag="mtmp")
                    nc.scalar.activation(mtmp[:], pm[:, :, 0:WM], AF.Exp,
                                         bias=bm_sb[:, q:q + 1])
                    nc.scalar.activation(mTv[:, q, :, 0:WM], mtmp[:],
                                         AF.Ln, bias=1.0)
                ctxa.close()

            # big decoder weights here: stream during the flux steps
            wdec_sb = per.tile([128, DC * VS], F32R)
            bdec_sb = per.tile([128, VS], F32)
            if do_dec:
                wd_i = nc.sync.dma_start(wdec_sb[:], t_wd.ap())
                nc.sync.dma_start(bdec_sb[:], t_bd.ap())
                if do_flux:
                    # keep the big stream out of the gathers' way
                    from concourse.bass import _add_dep_helper
                    _add_dep_helper(wd_i.ins, gather_insts[-1].ins, sync=True,
                                    reason="wdec stream after gathers")

            # ---- phase B: K flux steps ----
            if do_flux:
                with ExitStack() as ctxb:
                    pb = ctxb.enter_context(tc.tile_pool(name="pb", bufs=2))
                    pfl = ctxb.enter_context(tc.tile_pool(name="pfl", bufs=1))
                    hT = pfl.tile([128, E, WFP], F32R)
                    F_sb = pfl.tile([128, E, WFP], F32)
                    G_sb = pfl.tile([128, E, WFP], F32)
                    Gd_sb = pfl.tile([128, E, WF - 1], F32)
                    mupd = pfl.tile([128, E, WM - 2], F32)
                    hTv = hT[:].rearrange("p (b g) t -> p g b t", b=B)
                    Fv = F_sb[:].rearrange("p (b g) t -> p g b t", b=B)
                    for k in range(KSTEPS):
                        # h = tanh(m_l @ W1a + m_r @ W1b + b1)
                        for q in range(DC):
                            ph = psf.tile([128, 2, 512], F32, tag="ps", space="PSUM")
                            for gd in range(DC):
                                for sh in range(2):
                                    for bb in range(B):
                                        nc.tensor.matmul(
                                            ph[:, bb, 0:WFP],
                                            wf1_sb[:, sh, gd, q * 128:(q + 1) * 128],
                                            mT[:, bb * DC + gd, sh:sh + WFP],
                                            start=(gd == 0 and sh == 0),
                                            stop=(gd == DC - 1 and sh == 1),
                                        )
                            nc.scalar.activation(hTv[:, q, :, :], ph[:, :, 0:WFP],
                                                 AF.Tanh, bias=bf1_sb[:, q:q + 1])
                        # F = softplus(h @ W2 + b2)
                        for q in range(DC):
                            pf = psf.tile([128, 2, 512], F32, tag="ps", space="PSUM")
                            for gd in range(DC):
                                for bb in range(B):
                                    nc.tensor.matmul(
                                        pf[:, bb, 0:WFP],
                                        wf2_sb[:, gd, q * 128:(q + 1) * 128],
                                        hT[:, bb * DC + gd, :],
                                        start=(gd == 0), stop=(gd == DC - 1),
                                    )
                            ftmp = pb.tile([128, B, WFP], F32, tag="ftmp")
                            nc.scalar.activation(ftmp[:], pf[:, :, 0:WFP], AF.Exp,
                                                 bias=bf2_sb[:, q:q + 1])
                            nc.scalar.activation(Fv[:, q, :, :], ftmp[:],
                                                 AF.Ln, bias=1.0)
                        # G = dt*mask*F ; dm = G[j-1]-G[j]; m = max(m+dm, EPS)
                        # split per (batch, d-chunk): the update is elementwise
                        # over d, so each chunk's chain unblocks its next-step
                        # matmuls as soon as that chunk's F is ready
                        for bb in range(B):
                            for q in range(DC):
                                e0 = bb * DC + q
                                nc.vector.tensor_tensor(
                                    out=G_sb[:, e0:e0 + 1, :],
                                    in0=F_sb[:, e0:e0 + 1, :],
                                    in1=mdt_sb[:, None, :],
                                    op=mybir.AluOpType.mult,
                                )
                                nc.vector.tensor_tensor(
                                    out=Gd_sb[:, e0:e0 + 1, :],
                                    in0=G_sb[:, e0:e0 + 1, 0:WF - 1],
                                    in1=G_sb[:, e0:e0 + 1, 1:WF],
                                    op=mybir.AluOpType.subtract,
                                )
                                nc.vector.tensor_tensor(
                                    out=mupd[:, e0:e0 + 1, :],
                                    in0=mT_f32[:, e0:e0 + 1, 1:WM - 1],
                                    in1=Gd_sb[:, e0:e0 + 1, :],
                                    op=mybir.AluOpType.add,
                                )
                                nc.vector.tensor_scalar_max(
                                    mT[:, e0:e0 + 1, 1:WM - 1],
                                    mupd[:, e0:e0 + 1, :], EPS)

            ctxf.close()

            # ---- phase C: allgather m, 4-way split by (half, batch) so the
            # decoder starts after 1/4 of the payload and the rest pipelines
            # under the decode groups; each quarter keeps one batch's 4
            # contiguous d-chunks so lt reads stay single contiguous DMAs
            cc_outs = {}
            for hh in range(2):
                for bb in range(B):
                    cc_in = dram.tile([128, DC * 128], F32,
                                      name=f"cc_in{hh}{bb}")
                    cc_out = dram.tile([NCORES * 128, DC * 128], F32,
                                       name=f"cc_out{hh}{bb}")
                    if do_flux:
                        nc.sync.dma_start(
                            cc_in[:].rearrange("p (g t) -> p g t", g=DC),
                            mT_f32[:, bb * DC:(bb + 1) * DC,
                                   HALO + hh * 128: HALO + (hh + 1) * 128],
                        )
                    elif variant == "cc":
                        nc.sync.dma_start(cc_in[:, 0:WFP], mdt_sb[:])
                    if do_cc:
                        nc.gpsimd.collective_compute(
                            "AllGather", mybir.AluOpType.bypass,
                            replica_groups=[list(range(NCORES))],
                            ins=[cc_in[:]], outs=[cc_out[:]],
                        )
                    cc_outs[(hh, bb)] = cc_out[:].bitcast(F32R)
            ctxw.close()

            # ---- phase D: decoder (h=0 token tiles first, then h=1) ----
            if do_dec:
                with ExitStack() as ctxd:
                    psd_pool = ctxd.enter_context(
                        tc.tile_pool(name="psd", bufs=8, space="PSUM"))
                    pl = ctxd.enter_context(tc.tile_pool(name="pl", bufs=8))
                    po = ctxd.enter_context(tc.tile_pool(name="po", bufs=4))
                    order = [(hh, bb, r) for hh in range(2) for bb in range(B)
                             for r in range(NCORES)]
                    for hh, bb, r in order:
                        j = bb * 16 + 2 * r + hh
                        src = cc_outs[(hh, bb)]
                        # all 4 d-chunks of this token tile are contiguous
                        lt = pl.tile([128, DC * 128], F32R, tag="lt")
                        nc.sync.dma_start(
                            lt[:],
                            src[r * 128:(r + 1) * 128, :],
                        )
                        osb = po.tile([128, VS], F32, tag="osb")
                        for v in range(NV):
                            pd = psd_pool.tile([128, 512], F32, tag="psd")
                            for gd in range(DC):
                                nc.tensor.matmul(
                                    pd[:, 0:VT],
                                    lt[:, gd * 128:(gd + 1) * 128],
                                    wdec_sb[:, gd * VS + v * VT:
                                            gd * VS + (v + 1) * VT],
                                    start=(gd == 0), stop=(gd == DC - 1),
                                )
                            nc.vector.tensor_tensor(
                                out=osb[:, v * VT:(v + 1) * VT],
                                in0=pd[:, 0:VT],
                                in1=bdec_sb[:, v * VT:(v + 1) * VT],
                                op=mybir.AluOpType.add,
                            )
                        nc.sync.dma_start(
                            t_y.ap()[j * 128:(j + 1) * 128, :], osb[:])

    nc.compile()
    return nc


def _get_module(variant="all"):
    key = f"nc:{variant}"
    if key not in _CACHE:
        _CACHE[key] = _build_module(variant)
    return _CACHE[key]


def _prep_inputs(x, emb, w_mass, b_mass, w_f1, b_f1, w_f2, b_f2, cfl_raw,
                 w_dec, b_dec):
    x = np.asarray(x)
    emb = np.ascontiguousarray(np.asarray(emb, dtype=np.float32))
    w_mass = np.asarray(w_mass, dtype=np.float32)
    b_mass = np.asarray(b_mass, dtype=np.float32)
    w_f1 = np.asarray(w_f1, dtype=np.float32)
    b_f1 = np.asarray(b_f1, dtype=np.float32)
    w_f2 = np.asarray(w_f2, dtype=np.float32)
    b_f2 = np.asarray(b_f2, dtype=np.float32)
    w_dec = np.asarray(w_dec, dtype=np.float32)
    b_dec = np.asarray(b_dec, dtype=np.float32)
    dt = float(1.0 / (1.0 + np.exp(-np.float64(np.asarray(cfl_raw)))))

    wm_in = np.ascontiguousarray(
        w_mass.reshape(DC, 128, D).transpose(1, 0, 2).reshape(128, DC * D))
    wf1_in = np.ascontiguousarray(
        w_f1.reshape(2, DC, 128, D).transpose(2, 0, 1, 3).reshape(128, 2 * DC * D))
    wf2_in = np.ascontiguousarray(
        w_f2.reshape(DC, 128, D).transpose(1, 0, 2).reshape(128, DC * D))
    bm_in = np.ascontiguousarray(b_mass.reshape(DC, 128).T)
    bf1_in = np.ascontiguousarray(b_f1.reshape(DC, 128).T)
    bf2_in = np.ascontiguousarray(b_f2.reshape(DC, 128).T)

    in_maps = []
    for c in range(NCORES):
        sedge = c * SC - HALO
        idx = np.zeros(GT * 128, dtype=np.int32)
        for b in range(B):
            t = np.arange(WM)
            sc = np.clip(sedge + t, 0, S - 1)
            idx[b * WM:(b + 1) * WM] = x[b, sc]
        idx_t = np.ascontiguousarray(idx.reshape(GT, 128).T)

        j = np.arange(WFP)
        gp = sedge + j
        fm = ((gp >= 0) & (gp <= S - 2) & (j < WF)).astype(np.float32) \
            * np.float32(dt)
        mdt_in = np.ascontiguousarray(np.broadcast_to(fm, (128, WFP)))

        wd = w_dec[:, c * VS:(c + 1) * VS]
        wd_in = np.ascontiguousarray(
            wd.reshape(DC, 128, VS).transpose(1, 0, 2).reshape(128, DC * VS))
        bd_in = np.ascontiguousarray(
            np.broadcast_to(b_dec[c * VS:(c + 1) * VS], (128, VS)))

        in_maps.append({
            "t_idx": idx_t, "t_mdt": mdt_in,
            "t_wm": wm_in, "t_wf1": wf1_in, "t_wf2": wf2_in,
            "t_bm": bm_in, "t_bf1": bf1_in, "t_bf2": bf2_in,
            "t_emb": emb, "t_wd": wd_in, "t_bd": bd_in,
        })
    return in_maps


def kernel(**inputs) -> np.ndarray:
    global LAST_RESULTS
    nc = _get_module()
    in_maps = _prep_inputs(**inputs)
    res = run_bass_kernel_spmd(nc, in_maps, core_ids=list(range(NCORES)))
    LAST_RESULTS = res
    y = np.empty((B, S, V), dtype=np.float32)
    for c in range(NCORES):
        y[:, :, c * VS:(c + 1) * VS] = res.results[c]["t_y"].reshape(B, S, VS)
    return y


# revision 20
# speedup vs baseline: 6.5802x; 1.0027x over previous
"""Trainium2 Bass kernel for nn_DTFN (mass/flux stencil + vocab decoder).

Strategy (8 NeuronCores, SPMD single NEFF):
  - Sequence-parallel mass+flux: each core computes m for its S/8=256 seq
    positions using a K=3 halo of neighbors' initial state (no per-step
    exchange needed: halo depth K suffices for K local steps; global
    zero-flux boundaries handled by a per-core flux mask input).
  - AllGather the final m across the 8 cores, split into two halves so the
    decoder can start on the first half while the second is in flight.
  - Vocab-parallel decoder: each core computes logits for all 4096 tokens
    x its 4000-column shard of w_dec.
  All matmuls run as float32r (full PE rate at N>=256, ~13-bit mantissa).
  Layout is "transposed": d-chunks on partitions, tokens on the free axis,
  so flux-step outputs chain directly into the next matmul and into the
  decoder's stationary operand with no transposes after the initial one.
  State blocks are batch-major (e = b*DC + g) so each batch's flux update
  unblocks the next step's matmuls for that batch independently.
"""

import numpy as np
from contextlib import ExitStack

import concourse.bass as bass
import concourse.bacc as bacc
import concourse.mybir as mybir
import concourse.tile as tile
from concourse.bass_utils import run_bass_kernel_spmd
from concourse.masks import make_identity

F32 = mybir.dt.float32
F32R = mybir.dt.float32r
I32 = mybir.dt.int32
AF = mybir.ActivationFunctionType

V, D, KSTEPS, B, S = 32000, 512, 3, 2, 2048
EPS = 1e-6
NCORES = 8
SC = S // NCORES          # 256 seq positions per core
HALO = KSTEPS             # 3
WM = SC + 2 * HALO        # 262 m-cells per batch per core
WMA = WM + 2              # 264 allocated (2 zero-pad cells for even fp32r N)
WF = WM - 1               # 261 real flux pairs
WFP = WF + 1              # 262 computed pairs (fp32r needs even dst count)
NTOK = B * WM             # 524 gathered tokens per core
GT = (NTOK + 127) // 128  # 5 gather tiles (640 slots, rest padded)
DC = D // 128             # 4 d-chunks
E = DC * B                # 8 (batch, dchunk) blocks, b-major: e = b*DC + g
VS = V // NCORES          # 4000 vocab columns per core
NV = 8
VT = VS // NV             # 500
TJ = B * S // 128         # 32 token tiles of 128

_CACHE: dict = {}
LAST_RESULTS = None


def _build_module(variant="all"):
    nc = bacc.Bacc("TRN2", target_bir_lowering=False, debug=False,
                   num_devices=NCORES)

    do_flux = variant in ("all", "flux", "nocc2")
    do_dec = variant in ("all", "decoder", "nocc2")
    do_cc = variant in ("all", "cc")

    # --- per-core DRAM I/O ---
    t_idx = nc.dram_tensor("t_idx", [128, GT], I32, kind="ExternalInput")
    t_mdt = nc.dram_tensor("t_mdt", [128, WFP], F32, kind="ExternalInput")
    t_wm = nc.dram_tensor("t_wm", [128, DC * D], F32R, kind="ExternalInput")
    t_wf1 = nc.dram_tensor("t_wf1", [128, 2 * DC * D], F32R, kind="ExternalInput")
    t_wf2 = nc.dram_tensor("t_wf2", [128, DC * D], F32R, kind="ExternalInput")
    t_bm = nc.dram_tensor("t_bm", [128, DC], F32, kind="ExternalInput")
    t_bf1 = nc.dram_tensor("t_bf1", [128, DC], F32, kind="ExternalInput")
    t_bf2 = nc.dram_tensor("t_bf2", [128, DC], F32, kind="ExternalInput")
    t_emb = nc.dram_tensor("t_emb", [V, D], F32, kind="ExternalInput")
    t_wd = nc.dram_tensor("t_wd", [128, DC * VS], F32R, kind="ExternalInput")
    t_bd = nc.dram_tensor("t_bd", [128, VS], F32, kind="ExternalInput")
    t_y = nc.dram_tensor("t_y", [TJ * 128, VS], F32, kind="ExternalOutput")

    with tile.TileContext(nc) as tc:
        with ExitStack() as ctx:
            per = ctx.enter_context(tc.tile_pool(name="per", bufs=1))
            dram = ctx.enter_context(tc.tile_pool(name="dram", bufs=1, space="DRAM"))
            ctxw = ExitStack()
            pw = ctxw.enter_context(tc.tile_pool(name="pw", bufs=1))

            # ---- small loads first (so the gather isn't queued behind 10MB) ----
            idx_sb = pw.tile([128, GT], I32)
            nc.sync.dma_start(idx_sb[:], t_idx.ap())
            mdt_sb = pw.tile([128, WFP], F32)
            nc.sync.dma_start(mdt_sb[:], t_mdt.ap())
            bm_sb = pw.tile([128, DC], F32)
            nc.sync.dma_start(bm_sb[:], t_bm.ap())
            bf1_sb = pw.tile([128, DC], F32)
            nc.sync.dma_start(bf1_sb[:], t_bf1.ap())
            bf2_sb = pw.tile([128, DC], F32)
            nc.sync.dma_start(bf2_sb[:], t_bf2.ap())
            wm_sb = pw.tile([128, DC, D], F32R)
            nc.sync.dma_start(wm_sb[:], t_wm.ap().rearrange("p (g d) -> p g d", g=DC))
            wf1_sb = pw.tile([128, 2, DC, D], F32R)
            wf2_sb = pw.tile([128, DC, D], F32R)

            # persistent state: m transposed [p, (b g), t]
            mT = pw.tile([128, E, WMA], F32R)
            # zero the 2 pad cells (memset can't emit f32r; mul-by-0 can)
            nc.vector.tensor_scalar_mul(
                mT[:, :, WM:WMA],
                mdt_sb[:, None, 0:2].to_broadcast([128, E, 2]), 0.0)
            mT_f32 = mT[:].bitcast(F32)
            # b-major views [p, g, b, t]
            mTv = mT[:].rearrange("p (b g) t -> p g b t", b=B)
            mTv_f32 = mT_f32.rearrange("p (b g) t -> p g b t", b=B)

            # ---- phase A: gather + transpose ----
            ctxa = ExitStack()
            ctxf = ExitStack()
            if do_flux:
                psf = ctxf.enter_context(
                    tc.tile_pool(name="psf", bufs=4, space="PSUM"))
                pa = ctxa.enter_context(tc.tile_pool(name="pa", bufs=2))
                pe1 = ctxa.enter_context(tc.tile_pool(name="pe1", bufs=1))

                eT = pe1.tile([128, DC, GT * 128], F32R)
                ident = pe1.tile([128, 128], F32)
                make_identity(nc, ident[:])

                e_nats = []
                gather_insts = []
                for gt in range(GT):
                    e_nat = pa.tile([128, D], F32, tag="e_nat", bufs=3)
                    gi = nc.gpsimd.indirect_dma_start(
                        out=e_nat[:], out_offset=None,
                        in_=t_emb.ap(),
                        in_offset=bass.IndirectOffsetOnAxis(
                            ap=idx_sb[:, gt:gt + 1], axis=0),
                    )
                    gather_insts.append(gi)
                    e_nats.append(e_nat)

            if do_flux:
                for gt in range(GT):
                    for gd in range(DC):
                        tp = psf.tile([128, 2, 512], F32, tag="ps", space="PSUM")
                        nc.tensor.transpose(
                            tp[:, 0, 0:128],
                            e_nats[gt][:, gd * 128:(gd + 1) * 128],
                            ident[:])
                        nc.vector.tensor_copy(
                            eT[:, gd, gt * 128:(gt + 1) * 128], tp[:, 0, 0:128])

                # flux weights stream in while transposes/mass run, but
                # behind the gathers so they don't delay the mass inputs
                from concourse.bass import _add_dep_helper
                wf1_i = nc.sync.dma_start(
                    wf1_sb[:],
                    t_wf1.ap().rearrange("p (s g d) -> p s g d", s=2, g=DC),
                )
                wf2_i = nc.sync.dma_start(
                    wf2_sb[:],
                    t_wf2.ap().rearrange("p (g d) -> p g d", g=DC))
                _add_dep_helper(wf1_i.ins, gather_insts[-1].ins, sync=True,
                                reason="wf1 stream after gathers")
                _add_dep_helper(wf2_i.ins, gather_insts[-1].ins, sync=True,
                                reason="wf2 stream after gathers")

                # mass: m = softplus(e @ w_mass + b_m)   [softplus = Ln(1+Exp)]
                for q in range(DC):
                    pm = psf.tile([128, 2, 512], F32, tag="ps", space="PSUM")
                    for gd in range(DC):
                        for bb in range(B):
                            nc.tensor.matmul(
                                pm[:, bb, 0:WM],
                                wm_sb[:, gd, q * 128:(q + 1) * 128],
                                eT[:, gd, bb * WM:(bb + 1) * WM],
                                start=(gd == 0), stop=(gd == DC - 1),
                            )
                    mtmp = pa.tile([128, B, WM], F32, tag="mtmp")
                    nc.scalar.activation(mtmp[:], pm[:, :, 0:WM], AF.Exp,
                                         bias=bm_sb[:, q:q + 1])
                    nc.scalar.activation(mTv[:, q, :, 0:WM], mtmp[:],
                                         AF.Ln, bias=1.0)
                ctxa.close()

            # big decoder weights here: stream during the flux steps
            wdec_sb = per.tile([128, DC * VS], F32R)
            bdec_sb = per.tile([128, VS], F32)
            if do_dec:
                wd_i = nc.sync.dma_start(wdec_sb[:], t_wd.ap())
                nc.sync.dma_start(bdec_sb[:], t_bd.ap())
                if do_flux:
                    # keep the big stream out of the gathers' way
                    from concourse.bass import _add_dep_helper
                    _add_dep_helper(wd_i.ins, gather_insts[-1].ins, sync=True,
                                    reason="wdec stream after gathers")

            # ---- phase B: K flux steps ----
            if do_flux:
                with ExitStack() as ctxb:
                    pb = ctxb.enter_context(tc.tile_pool(name="pb", bufs=2))
                    pfl = ctxb.enter_context(tc.tile_pool(name="pfl", bufs=1))
                    hT = pfl.tile([128, E, WFP], F32R)
                    F_sb = pfl.tile([128, E, WFP], F32)
                    G_sb = pfl.tile([128, E, WFP], F32)
                    Gd_sb = pfl.tile([128, E, WF - 1], F32)
                    mupd = pfl.tile([128, E, WM - 2], F32)
                    hTv = hT[:].rearrange("p (b g) t -> p g b t", b=B)
                    Fv = F_sb[:].rearrange("p (b g) t -> p g b t", b=B)
                    for k in range(KSTEPS):
                        # h = tanh(m_l @ W1a + m_r @ W1b + b1)
                        for q in range(DC):
                            ph = psf.tile([128, 2, 512], F32, tag="ps", space="PSUM")
                            for gd in range(DC):
                                for sh in range(2):
                                    for bb in range(B):
                                        nc.tensor.matmul(
                                            ph[:, bb, 0:WFP],
                                            wf1_sb[:, sh, gd, q * 128:(q + 1) * 128],
                                            mT[:, bb * DC + gd, sh:sh + WFP],
                                            start=(gd == 0 and sh == 0),
                                            stop=(gd == DC - 1 and sh == 1),
                                        )
                            nc.scalar.activation(hTv[:, q, :, :], ph[:, :, 0:WFP],
                                                 AF.Tanh, bias=bf1_sb[:, q:q + 1])
                        # F = softplus(h @ W2 + b2)
                        for q in range(DC):
                            pf = psf.tile([128, 2, 512], F32, tag="ps", space="PSUM")
                            for gd in range(DC):
                                for bb in range(B):
                                    nc.tensor.matmul(
                                        pf[:, bb, 0:WFP],
                                        wf2_sb[:, gd, q * 128:(q + 1) * 128],
                                        hT[:, bb * DC + gd, :],
                                        start=(gd == 0), stop=(gd == DC - 1),
                                    )
                            ftmp = pb.tile([128, B, WFP], F32, tag="ftmp")
                            nc.scalar.activation(ftmp[:], pf[:, :, 0:WFP], AF.Exp,
                                                 bias=bf2_sb[:, q:q + 1])
                            nc.scalar.activation(Fv[:, q, :, :], ftmp[:],
                                                 AF.Ln, bias=1.0)
                        # G = dt*mask*F ; dm = G[j-1]-G[j]; m = max(m+dm, EPS)
                        # split per (batch, d-chunk): the update is elementwise
                        # over d, so each chunk's chain unblocks its next-step
                        # matmuls as soon as that chunk's F is ready
                        for bb in range(B):
                            for q in range(DC):
                                e0 = bb * DC + q
                                nc.vector.tensor_tensor(
                                    out=G_sb[:, e0:e0 + 1, :],
                                    in0=F_sb[:, e0:e0 + 1, :],
                                    in1=mdt_sb[:, None, :],
                                    op=mybir.AluOpType.mult,
                                )
                                nc.vector.tensor_tensor(
                                    out=Gd_sb[:, e0:e0 + 1, :],
                                    in0=G_sb[:, e0:e0 + 1, 0:WF - 1],
                                    in1=G_sb[:, e0:e0 + 1, 1:WF],
                                    op=mybir.AluOpType.subtract,
                                )
                                nc.vector.tensor_tensor(
                                    out=mupd[:, e0:e0 + 1, :],
                                    in0=mT_f32[:, e0:e0 + 1, 1:WM - 1],
                                    in1=Gd_sb[:, e0:e0 + 1, :],
                                    op=mybir.AluOpType.add,
                                )
                                nc.vector.tensor_scalar_max(
                                    mT[:, e0:e0 + 1, 1:WM - 1],
                                    mupd[:, e0:e0 + 1, :], EPS)

            ctxf.close()

            # ---- phase C: allgather m, 4-way split by (half, batch) so the
            # decoder starts after 1/4 of the payload and the rest pipelines
            # under the decode groups; each quarter keeps one batch's 4
            # contiguous d-chunks so lt reads stay single contiguous DMAs
            cc_outs = {}
            for hh in range(2):
                for bb in range(B):
                    cc_in = dram.tile([128, DC * 128], F32,
                                      name=f"cc_in{hh}{bb}")
                    cc_out = dram.tile([NCORES * 128, DC * 128], F32,
                                       name=f"cc_out{hh}{bb}")
                    if do_flux:
                        nc.sync.dma_start(
                            cc_in[:].rearrange("p (g t) -> p g t", g=DC),
                            mT_f32[:, bb * DC:(bb + 1) * DC,
                                   HALO + hh * 128: HALO + (hh + 1) * 128],
                        )
                    elif variant == "cc":
                        nc.sync.dma_start(cc_in[:, 0:WFP], mdt_sb[:])
                    if do_cc:
                        nc.gpsimd.collective_compute(
                            "AllGather", mybir.AluOpType.bypass,
                            replica_groups=[list(range(NCORES))],
                            ins=[cc_in[:]], outs=[cc_out[:]],
                        )
                    cc_outs[(hh, bb)] = cc_out[:].bitcast(F32R)
            ctxw.close()

            # ---- phase D: decoder (h=0 token tiles first, then h=1) ----
            if do_dec:
                with ExitStack() as ctxd:
                    psd_pool = ctxd.enter_context(
                        tc.tile_pool(name="psd", bufs=8, space="PSUM"))
                    pl = ctxd.enter_context(tc.tile_pool(name="pl", bufs=8))
                    po = ctxd.enter_context(tc.tile_pool(name="po", bufs=4))
                    order = [(hh, bb, r) for hh in range(2) for bb in range(B)
                             for r in range(NCORES)]
                    for hh, bb, r in order:
                        j = bb * 16 + 2 * r + hh
                        src = cc_outs[(hh, bb)]
                        # all 4 d-chunks of this token tile are contiguous
                        lt = pl.tile([128, DC * 128], F32R, tag="lt")
                        nc.sync.dma_start(
                            lt[:],
                            src[r * 128:(r + 1) * 128, :],
                        )
                        osb = po.tile([128, VS], F32, tag="osb")
                        for v in range(NV):
                            pd = psd_pool.tile([128, 512], F32, tag="psd")
                            for gd in range(DC):
                                nc.tensor.matmul(
                                    pd[:, 0:VT],
                                    lt[:, gd * 128:(gd + 1) * 128],
                                    wdec_sb[:, gd * VS + v * VT:
                                            gd * VS + (v + 1) * VT],
                                    start=(gd == 0), stop=(gd == DC - 1),
                                )
                            nc.vector.tensor_tensor(
                                out=osb[:, v * VT:(v + 1) * VT],
                                in0=pd[:, 0:VT],
                                in1=bdec_sb[:, v * VT:(v + 1) * VT],
                                op=mybir.AluOpType.add,
                            )
                        nc.sync.dma_start(
                            t_y.ap()[j * 128:(j + 1) * 128, :], osb[:])

    nc.compile()
    return nc


def _get_module(variant="all"):
    key = f"nc:{variant}"
    if key not in _CACHE:
        _CACHE[key] = _build_module(variant)
    return _CACHE[key]


def _prep_inputs(x, emb, w_mass, b_mass, w_f1, b_f1, w_f2, b_f2, cfl_raw,
                 w_dec, b_dec):
    x = np.asarray(x)
    emb = np.ascontiguousarray(np.asarray(emb, dtype=np.float32))
    w_mass = np.asarray(w_mass, dtype=np.float32)
    b_mass = np.asarray(b_mass, dtype=np.float32)
    w_f1 = np.asarray(w_f1, dtype=np.float32)
    b_f1 = np.asarray(b_f1, dtype=np.float32)
    w_f2 = np.asarray(w_f2, dtype=np.float32)
    b_f2 = np.asarray(b_f2, dtype=np.float32)
    w_dec = np.asarray(w_dec, dtype=np.float32)
    b_dec = np.asarray(b_dec, dtype=np.float32)
    dt = float(1.0 / (1.0 + np.exp(-np.float64(np.asarray(cfl_raw)))))

    wm_in = np.ascontiguousarray(
        w_mass.reshape(DC, 128, D).transpose(1, 0, 2).reshape(128, DC * D))
    wf1_in = np.ascontiguousarray(
        w_f1.reshape(2, DC, 128, D).transpose(2, 0, 1, 3).reshape(128, 2 * DC * D))
    wf2_in = np.ascontiguousarray(
        w_f2.reshape(DC, 128, D).transpose(1, 0, 2).reshape(128, DC * D))
    bm_in = np.ascontiguousarray(b_mass.reshape(DC, 128).T)
    bf1_in = np.ascontiguousarray(b_f1.reshape(DC, 128).T)
    bf2_in = np.ascontiguousarray(b_f2.reshape(DC, 128).T)

    in_maps = []
    for c in range(NCORES):
        sedge = c * SC - HALO
        idx = np.zeros(GT * 128, dtype=np.int32)
        for b in range(B):
            t = np.arange(WM)
            sc = np.clip(sedge + t, 0, S - 1)
            idx[b * WM:(b + 1) * WM] = x[b, sc]
        idx_t = np.ascontiguousarray(idx.reshape(GT, 128).T)

        j = np.arange(WFP)
        gp = sedge + j
        fm = ((gp >= 0) & (gp <= S - 2) & (j < WF)).astype(np.float32) \
            * np.float32(dt)
        mdt_in = np.ascontiguousarray(np.broadcast_to(fm, (128, WFP)))

        wd = w_dec[:, c * VS:(c + 1) * VS]
        wd_in = np.ascontiguousarray(
            wd.reshape(DC, 128, VS).transpose(1, 0, 2).reshape(128, DC * VS))
        bd_in = np.ascontiguousarray(
            np.broadcast_to(b_dec[c * VS:(c + 1) * VS], (128, VS)))

        in_maps.append({
            "t_idx": idx_t, "t_mdt": mdt_in,
            "t_wm": wm_in, "t_wf1": wf1_in, "t_wf2": wf2_in,
            "t_bm": bm_in, "t_bf1": bf1_in, "t_bf2": bf2_in,
            "t_emb": emb, "t_wd": wd_in, "t_bd": bd_in,
        })
    return in_maps


def kernel(**inputs) -> np.ndarray:
    global LAST_RESULTS
    nc = _get_module()
    in_maps = _prep_inputs(**inputs)
    res = run_bass_kernel_spmd(nc, in_maps, core_ids=list(range(NCORES)))
    LAST_RESULTS = res
    y = np.empty((B, S, V), dtype=np.float32)
    for c in range(NCORES):
        y[:, :, c * VS:(c + 1) * VS] = res.results[c]["t_y"].reshape(B, S, VS)
    return y
